# revision 1
# baseline (speedup 1.0000x reference)
"""Qwen2.5-VL attention (mrope + GQA + causal mask + o_proj) on 8 Trainium2
NeuronCores.

Sharding: batch x query-chunk. Core c handles batch b = c//4 and query rows
[512*(c%4), 512*(c%4)+512). Each core computes K/V projections for all 2048
tokens of its batch, Q projection + full attention + o_proj for its 512 query
rows, and writes a [512, 2048] output slice. Host concatenates - no
cross-core reduction.

On-device layout: everything transposed so the PE contraction dim is always
on partitions.  Host pre-transposes hidden (xT), weights (wqT/wkT/wvT/woT),
merged-mrope cos/sin, and the mask slice.
  - QT/KT produced as [d, t]; scores computed transposed S^T[k, q]
  - exp on ScalarE straight from PSUM with the 1/sqrt(D) scale folded in;
    additive mask applied as elementwise multiply by host-precomputed
    exp(mask) (exact 0/1 for a causal mask)
  - softmax denominators via ones[128,128] matmuls (sums arrive broadcast
    across partitions), normalization = reciprocal + multiply
  - PV accumulates outT[d, q]; o_proj consumes outT directly as lhsT
  - Q projection is interleaved with attention per head group so the wq
    weight stream hides behind attention compute

Matmuls run in fp32r (fp32 with 12-bit mantissa rounding, 4x faster than
plain fp32 on the PE).  Host pre-rounds all DMA-fed matmul operands; compute
ops that produce matmul operands write fp32r tiles (HW rounds on write).
"""

import sys

for _p in ("/opt/trn_rl_repo", "/root/.axon_site/_ro/trn_rl_repo"):
    if _p not in sys.path:
        sys.path.insert(0, _p)

import numpy as np

B = 2
S = 2048
HID = 2048
NH = 16
NKV = 2
D = 128
NQ = 512          # query rows per core
N_CORES = 8
SM_SCALE = 1.0 / np.sqrt(np.float32(D))

_BUILD_CACHE = {}


def _round_fp32r(a):
    """Round-to-nearest-even to 12 explicit mantissa bits (fp32r)."""
    u = np.ascontiguousarray(a, np.float32).view(np.uint32)
    low = u & np.uint32(0xFFF)
    up = (u & np.uint32(0xFFFFF000)) + np.uint32(0x1000)
    half = low == np.uint32(0x800)
    rnd = np.where(low > 0x800, up,
                   np.where(half & ((u & np.uint32(0x1000)) != 0), up,
                            u & np.uint32(0xFFFFF000)))
    expmask = (u & np.uint32(0x7F800000)) == np.uint32(0x7F800000)
    rnd = np.where(expmask, u, rnd)
    return rnd.view(np.float32)


def _build_nc(mm="f32r"):
    import contextlib
    import concourse.bass as bass
    import concourse.tile as tile
    from concourse import bacc, mybir

    F32 = mybir.dt.float32
    MMDT = mybir.dt.float32r if mm == "f32r" else F32

    nc = bacc.Bacc(target_bir_lowering=False, debug=False)

    def param(name, shape, dt=MMDT):
        return nc.declare_dram_parameter(name, list(shape), dt,
                                         isOutput=False)[:]

    xT = param("xT", [HID, S])
    wqT = param("wqT", [HID, HID])
    wkT = param("wkT", [HID, NKV * D])
    wvT = param("wvT", [HID, NKV * D])
    woT = param("woT", [HID, HID])
    bqT_d = param("bqT", [D, NH], F32)
    bkT_d = param("bkT", [D, NKV], F32)
    bv_d = param("bv", [1, NKV * D])
    cosT_d = param("cosT", [D, S])
    sinT_d = param("sinT", [D, S])
    cq_d = param("cosTq", [D, NQ])
    sq_d = param("sinTq", [D, NQ])
    maskT_d = param("maskT", [S, NQ])     # exp(mask).T, fp32r-rounded
    out_d = nc.declare_dram_parameter("out", [NQ, HID], F32, isOutput=True)[:]

    HC = HID // 128   # 16 contraction chunks
    KT = S // 128     # 16 key tiles
    KT2 = KT // 2     # 8 key tile-pairs
    TC = S // NQ      # 4 token chunks (for K/V proj)
    QS = NQ // 128    # 4 query sub-tiles

    Exp = mybir.ActivationFunctionType.Exp
    Ident = mybir.ActivationFunctionType.Identity

    lp = (nc.allow_low_precision(reason="fp32r matmul operands; psum stays f32")
          if mm == "f32r" else contextlib.nullcontext())
    with lp, tile.TileContext(nc) as tc:
        with tc.tile_pool(name="const", bufs=1) as cst, \
             tc.tile_pool(name="maskp", bufs=1) as maskp, \
             tc.tile_pool(name="kvp", bufs=1) as kvp:

            ones_row = cst.tile([1, 128], MMDT, name="ones_row")
            ones_sq = cst.tile([128, 128], MMDT, name="ones_sq")
            ones_f32 = cst.tile([128, 128], F32, name="ones_f32")
            nc.vector.memset(ones_f32, 1.0)
            nc.vector.tensor_copy(ones_row, ones_f32[0:1, :])
            nc.vector.tensor_copy(ones_sq, ones_f32)
            bqT = cst.tile([D, NH], F32, name="bqT")
            bkT = cst.tile([D, NKV], F32, name="bkT")
            bvr = cst.tile([1, NKV * D], MMDT, name="bvr")
            nc.sync.dma_start(bqT, bqT_d)
            nc.sync.dma_start(bkT, bkT_d)
            nc.sync.dma_start(bvr, bv_d)

            # exp(mask) tiles [128 k, 2 kt, 512 q], resident through attention
            mask_sb = [maskp.tile([128, 2, NQ], MMDT, name=f"mask{kt}")
                       for kt in range(KT2)]

            # token chunk 0 of xT = this core's query columns (host permutes
            # chunks); kept resident for the Q projection
            xq_sb = [kvp.tile([128, NQ], MMDT, name=f"xq{c}")
                     for c in range(HC)]
            # persistent K^T [d, t] per kv head; V [t, d] per token tile
            kT_sb = [kvp.tile([128, S], MMDT, name=f"kT{g}")
                     for g in range(NKV)]
            v_sb = [kvp.tile([128, NKV * D], MMDT, name=f"v{t}")
                    for t in range(KT)]

            # ---------------- P1a: K/V projection over all tokens ----------
            with tc.tile_pool(name="p1", bufs=1) as p1, \
                 tc.tile_pool(name="p1s", bufs=3) as p1s, \
                 tc.tile_pool(name="p1ps", bufs=1, space="PSUM") as p1ps:
                wk_sb = [p1.tile([128, NKV * D], MMDT, name=f"wk{c}")
                         for c in range(HC)]
                wv_sb = [p1.tile([128, NKV * D], MMDT, name=f"wv{c}")
                         for c in range(HC)]

                for tch in range(TC):
                    tsl = slice(tch * NQ, (tch + 1) * NQ)
                    kps = [p1ps.tile([128, NQ], F32, name=f"kps{g}", bufs=2)
                           for g in range(NKV)]
                    vps = [p1ps.tile([128, NKV * D], F32, name=f"vps{s_}",
                                     bufs=1) for s_ in range(4)]
                    for c in range(HC):
                        if tch == 0:
                            nc.sync.dma_start(wk_sb[c],
                                              wkT[c * 128:(c + 1) * 128, :])
                            nc.sync.dma_start(wv_sb[c],
                                              wvT[c * 128:(c + 1) * 128, :])
                            xt = xq_sb[c]
                        else:
                            xt = p1s.tile([128, NQ], MMDT, name="xt",
                                          bufs=8)
                        nc.sync.dma_start(xt, xT[c * 128:(c + 1) * 128, tsl])
                        for g in range(NKV):
                            nc.tensor.matmul(
                                kps[g], wk_sb[c][:, g * D:(g + 1) * D],
                                xt, start=(c == 0), stop=(c == HC - 1))
                        for s_ in range(4):
                            nc.tensor.matmul(
                                vps[s_], xt[:, s_ * 128:(s_ + 1) * 128],
                                wv_sb[c], start=(c == 0), stop=False)
                    # V bias via K=1 ones matmul, then evacuate
                    for s_ in range(4):
                        nc.tensor.matmul(vps[s_], ones_row, bvr,
                                         start=False, stop=True)
                        nc.vector.tensor_copy(v_sb[tch * 4 + s_], vps[s_])
                    # K bias + rope -> kT_sb
                    csb = p1s.tile([128, NQ], MMDT, name="csb")
                    ssb = p1s.tile([128, NQ], MMDT, name="ssb")
                    nc.sync.dma_start(csb, cosT_d[:, tsl])
                    nc.sync.dma_start(ssb, sinT_d[:, tsl])
                    for g in range(NKV):
                        kb = p1s.tile([128, NQ], MMDT, name="kb")
                        nc.scalar.activation(kb, kps[g], Ident,
                                             bias=bkT[:, g:g + 1])
                        ke = kT_sb[g][:, tsl]
                        shuf = p1s.tile([128, NQ], MMDT, name="shuf")
                        nc.sync.dma_start(shuf[0:64, :], kb[64:128, :])
                        nc.sync.dma_start(shuf[64:128, :], kb[0:64, :])
                        nc.vector.tensor_mul(ke, kb, csb)
                        nc.vector.tensor_mul(shuf, shuf, ssb)
                        nc.vector.tensor_add(ke, ke, shuf)

            # -------- P1b + P2: Q proj interleaved with attention ----------
            with tc.tile_pool(name="ap", bufs=1) as ap:
                a_sb = [ap.tile([128, NQ], MMDT, name=f"a{h}")
                        for h in range(NH)]
                with tc.tile_pool(name="p2", bufs=1) as p2, \
                     tc.tile_pool(name="p2s", bufs=2) as p2s, \
                     tc.tile_pool(name="p2w", bufs=10) as p2w, \
                     tc.tile_pool(name="qtp", bufs=2) as qtp, \
                     tc.tile_pool(name="att", bufs=5) as att, \
                     tc.tile_pool(name="atts", bufs=2) as atts:
                    cq = p2.tile([D, NQ], MMDT, name="cq")
                    sq = p2.tile([D, NQ], MMDT, name="sq")
                    nc.sync.dma_start(cq, cq_d)
                    nc.sync.dma_start(sq, sq_d)

                    for hg in range(4):
                        qT_sb = {}
                        with tc.tile_pool(name=f"qps{hg}", bufs=1,
                                          space="PSUM") as p2ps:
                            qps = [p2ps.tile([128, NQ], F32, name=f"qps{j}",
                                             bufs=1) for j in range(4)]
                            for c in range(HC):
                                wq = p2w.tile([128, NQ], MMDT, name="wq")
                                nc.sync.dma_start(
                                    wq, wqT[c * 128:(c + 1) * 128,
                                            hg * NQ:(hg + 1) * NQ])
                                for j in range(4):
                                    nc.tensor.matmul(
                                        qps[j], wq[:, j * 128:(j + 1) * 128],
                                        xq_sb[c], start=(c == 0),
                                        stop=(c == HC - 1))
                            for j in range(4):
                                h = hg * 4 + j
                                qT_sb[h] = qtp.tile([128, NQ], MMDT,
                                                    name=f"qT{j}")
                                qb = p2s.tile([128, NQ], MMDT, name="qb")
                                nc.scalar.activation(qb, qps[j], Ident,
                                                     bias=bqT[:, h:h + 1])
                                qe = qT_sb[h]
                                shufq = p2s.tile([128, NQ], MMDT,
                                                 name="shufq")
                                nc.sync.dma_start(shufq[0:64, :],
                                                  qb[64:128, :])
                                nc.sync.dma_start(shufq[64:128, :],
                                                  qb[0:64, :])
                                nc.vector.tensor_mul(qe, qb, cq)
                                nc.vector.tensor_mul(shufq, shufq, sq)
                                nc.vector.tensor_add(qe, qe, shufq)

                        if hg == 0:
                            for kt2 in range(KT2):
                                nc.sync.dma_start(
                                    mask_sb[kt2],
                                    maskT_d[256 * kt2:256 * (kt2 + 1),
                                            :].rearrange(
                                        "(a p) q -> p a q", a=2))
                        with tc.tile_pool(name=f"attps{hg}", bufs=1,
                                          space="PSUM") as attps:
                            for h in range(hg * 4, hg * 4 + 4):
                                g = h // (NH // NKV)
                                ops = attps.tile([128, NQ], F32, name="ops",
                                                 bufs=1)
                                stats = attps.tile([128, NQ], F32,
                                                   name="stats", bufs=1)
                                for kt2 in range(KT2):
                                    sps = attps.tile([128, 2, NQ], F32,
                                                     name="sps", bufs=3)
                                    ebuf = att.tile([128, 2, NQ], MMDT,
                                                    name="ebuf")
                                    for j2 in range(2):
                                        kt = 2 * kt2 + j2
                                        nc.tensor.matmul(
                                            sps[:, j2, :],
                                            kT_sb[g][:, kt * 128:
                                                     (kt + 1) * 128],
                                            qT_sb[h], start=True, stop=True)
                                    nc.scalar.activation(
                                        ebuf.rearrange("p a b -> p (a b)"),
                                        sps.rearrange("p a b -> p (a b)"),
                                        Exp, scale=float(SM_SCALE))
                                    nc.vector.tensor_mul(
                                        ebuf.rearrange("p a b -> p (a b)"),
                                        ebuf.rearrange("p a b -> p (a b)"),
                                        mask_sb[kt2].rearrange(
                                            "p a b -> p (a b)"))
                                    for j2 in range(2):
                                        kt = 2 * kt2 + j2
                                        nc.tensor.matmul(
                                            stats, ones_sq, ebuf[:, j2, :],
                                            start=(kt == 0),
                                            stop=(kt == KT - 1))
                                        nc.tensor.matmul(
                                            ops,
                                            v_sb[kt][:, g * D:(g + 1) * D],
                                            ebuf[:, j2, :],
                                            start=(kt == 0),
                                            stop=(kt == KT - 1))
                                recip = atts.tile([128, NQ], F32,
                                                  name="recip")
                                nc.vector.reciprocal_approx_fast(
                                    out=recip, in_=stats)
                                nc.vector.tensor_mul(a_sb[h], ops, recip)

                # ------------- P3: o_proj ------------------------------
                with tc.tile_pool(name="wop", bufs=1) as wop, \
                     tc.tile_pool(name="wos", bufs=3) as wos, \
                     tc.tile_pool(name="wops", bufs=1, space="PSUM") as wops:
                    for ec in range(4):
                        wo_t = [wop.tile([128, NQ], MMDT, name=f"wo{h}",
                                         bufs=2) for h in range(NH)]
                        for h in range(NH):
                            nc.sync.dma_start(
                                wo_t[h], woT[h * 128:(h + 1) * 128,
                                             ec * NQ:(ec + 1) * NQ])
                        for qs_ in range(QS):
                            opo = wops.tile([128, NQ], F32, name="opo",
                                            bufs=3)
                            for h in range(NH):
                                nc.tensor.matmul(
                                    opo,
                                    a_sb[h][:, qs_ * 128:(qs_ + 1) * 128],
                                    wo_t[h], start=(h == 0),
                                    stop=(h == NH - 1))
                            osb = wos.tile([128, NQ], F32, name="osb")
                            nc.vector.tensor_copy(osb, opo)
                            nc.sync.dma_start(
                                out_d[qs_ * 128:(qs_ + 1) * 128,
                                      ec * NQ:(ec + 1) * NQ], osb)
    return nc


def get_nc(mm="f32r"):
    if mm not in _BUILD_CACHE:
        nc = _build_nc(mm)
        nc.finalize()
        _BUILD_CACHE[mm] = nc
    return _BUILD_CACHE[mm]


_MROPE_SECTION = [16, 24, 24]
_STREAM_IDX = np.concatenate(
    [np.full(n, i % 3, np.int64)
     for i, n in enumerate(_MROPE_SECTION * 2)])  # [128]


def _host_prep(hidden_states, cos, sin, attention_mask, Wq, bq, Wk, bk, Wv,
               bv, Wo, mm="f32r"):
    f = np.float32
    if mm == "f32r":
        rnd = _round_fp32r
    else:
        def rnd(a):
            return np.ascontiguousarray(a, f)
    hs = np.asarray(hidden_states, f)
    cos = np.asarray(cos, f)
    sin = np.asarray(sin, f)
    mask = np.asarray(attention_mask, f)
    ar = np.arange(D)

    shared = {
        "wqT": rnd(np.asarray(Wq, f).T),
        "wkT": rnd(np.asarray(Wk, f).T),
        "wvT": rnd(np.asarray(Wv, f).T),
        "woT": rnd(np.asarray(Wo, f).T),
        "bqT": np.ascontiguousarray(np.asarray(bq, f).reshape(NH, D).T),
        "bkT": np.ascontiguousarray(np.asarray(bk, f).reshape(NKV, D).T),
        "bv": rnd(np.asarray(bv, f).reshape(1, NKV * D)),
    }

    per_batch = []
    for b in range(B):
        xT = rnd(hs[b].T)
        cosT = rnd(cos[_STREAM_IDX, b, :, ar])  # [128, S]
        sinT = rnd(sin[_STREAM_IDX, b, :, ar])
        sinT[0:64, :] *= -1.0   # rotate_half sign folded into sin
        maskT = rnd(np.exp(mask[b, 0].T.astype(np.float64)
                           ).astype(np.float32))
        per_batch.append((xT, cosT, sinT, maskT))

    in_maps = []
    for c in range(N_CORES):
        b, qc = divmod(c, N_CORES // B)
        xT, cosT, sinT, maskT = per_batch[b]
        qsl = slice(qc * NQ, (qc + 1) * NQ)
        order = [qc] + [o for o in range(N_CORES // B) if o != qc]
        tperm = np.concatenate([np.arange(o * NQ, (o + 1) * NQ)
                                for o in order])
        m = dict(shared)
        m["xT"] = np.ascontiguousarray(xT[:, tperm])
        m["cosT"] = np.ascontiguousarray(cosT[:, tperm])
        m["sinT"] = np.ascontiguousarray(sinT[:, tperm])
        m["maskT"] = np.ascontiguousarray(maskT[tperm][:, qsl])
        m["cosTq"] = np.ascontiguousarray(cosT[:, qsl])
        m["sinTq"] = np.ascontiguousarray(sinT[:, qsl])
        in_maps.append(m)
    return in_maps


def kernel(hidden_states, cos, sin, attention_mask, Wq, bq, Wk, bk, Wv, bv,
           Wo, _trace=False, _mm="f32r"):
    from concourse.bass_utils import run_bass_kernel_spmd

    in_maps = _host_prep(hidden_states, cos, sin, attention_mask, Wq, bq, Wk,
                         bk, Wv, bv, Wo, mm=_mm)
    nc = get_nc(_mm)
    res = run_bass_kernel_spmd(nc, in_maps, list(range(N_CORES)),
                               trace=_trace)
    out = np.empty((B, S, HID), np.float32)
    for c in range(N_CORES):
        b, qc = divmod(c, N_CORES // B)
        out[b, qc * NQ:(qc + 1) * NQ, :] = res.results[c]["out"]
    kernel._last_results = res
    return out



# revision 9
# speedup vs baseline: 1.2633x; 1.2633x over previous
"""Qwen2.5-VL attention (mrope + GQA + causal mask + o_proj) on 8 Trainium2
NeuronCores.

Fast path (causal mask detected): batch x interleaved-query sharding.
Core c handles batch b = c//4 and the stride-4 query comb {c4, c4+4, ...}
(c4 = c%4).  Host permutes tokens within every aligned group of 4 so the
core's queries sit at on-device columns 0::4; key tiles of 128 then cover
the same global token sets on every core, giving an SPMD-uniform causal
structure: query sub-tile j (128 rows, global span [512j, 512j+512)) only
attends key tiles kt < 4(j+1).

Attention loop per head: 12 "main" key tiles with shrinking query windows
(512/384/256 wide, always >=256 so fp32r/bf16 matmuls stay 1 cycle/row)
plus a 4-tile diagonal "tail" computed jointly for a head PAIR (the two
heads' last 128-query windows concatenated to free dim 256).  Only the
single diagonal key tile per iteration needs the 0/1 mask multiply; fully
masked tiles are skipped, fully visible tiles skip the multiply.

Softmax denominators via ones[128,128] matmuls accumulated region-wise in
PSUM alongside PV (start/stop flags per region, skip_group_check).

dtypes: projections stream bf16 (x, Wq, Wk, Wv, Wo); rope + scores run in
fp32r (cos/sin, qT, kT tiles); exp output / V / attn weights bf16; all
PSUM accumulation f32.  Measured rel err ~6e-3 vs the f32 reference.

Fallback (non-causal mask): the original dense kernel (exp(mask)
multiplicative masking over all tiles, fp32r everywhere).
"""

import sys

for _p in ("/opt/trn_rl_repo", "/root/.axon_site/_ro/trn_rl_repo"):
    if _p not in sys.path:
        sys.path.insert(0, _p)

import numpy as np

B = 2
S = 2048
HID = 2048
NH = 16
NKV = 2
D = 128
NQ = 512          # query rows per core
N_CORES = 8
SM_SCALE = 1.0 / np.sqrt(np.float32(D))
NEG = -3.4028235e38

_BUILD_CACHE = {}


def _round_fp32r(a):
    """Round-to-nearest-even to 12 explicit mantissa bits (fp32r)."""
    u = np.ascontiguousarray(a, np.float32).view(np.uint32)
    low = u & np.uint32(0xFFF)
    up = (u & np.uint32(0xFFFFF000)) + np.uint32(0x1000)
    half = low == np.uint32(0x800)
    rnd = np.where(low > 0x800, up,
                   np.where(half & ((u & np.uint32(0x1000)) != 0), up,
                            u & np.uint32(0xFFFFF000)))
    expmask = (u & np.uint32(0x7F800000)) == np.uint32(0x7F800000)
    rnd = np.where(expmask, u, rnd)
    return rnd.view(np.float32)


def _bf16(a):
    import ml_dtypes
    return np.asarray(a, np.float32).astype(ml_dtypes.bfloat16)


# ---------------------------------------------------------------------------
# causal fast path
# ---------------------------------------------------------------------------

def _build_causal():
    import concourse.bass as bass  # noqa: F401 (env init)
    import concourse.tile as tile
    from concourse import bacc, mybir

    F32 = mybir.dt.float32
    F32R = mybir.dt.float32r
    BF16 = mybir.dt.bfloat16

    nc = bacc.Bacc(target_bir_lowering=False, debug=False)

    def param(name, shape, dt):
        return nc.declare_dram_parameter(name, list(shape), dt,
                                         isOutput=False)[:]

    xT_d = param("xT", [HID, S], BF16)          # permuted token order
    wqP_d = param("wqP", [HID, HID], BF16)      # [h*128+k, c*128+m]
    wkT_d = param("wkT", [HID, NKV * D], BF16)
    wvT_d = param("wvT", [HID, NKV * D], BF16)
    woT_d = param("woT", [HID, HID], BF16)
    bqT_d = param("bqT", [D, NH], F32)
    bkT_d = param("bkT", [D, NKV], F32)
    bv_d = param("bv", [1, NKV * D], BF16)
    cosT_d = param("cosT", [D, S], F32R)        # permuted, mrope-merged
    sinT_d = param("sinT", [D, S], F32R)        # top half sign-flipped
    maskq_d = param("maskq", [D, 4 * D], BF16)  # diag band 0/1, j-invariant
    out_d = nc.declare_dram_parameter("out", [NQ, HID], F32, isOutput=True)[:]

    HC = HID // 128   # 16 contraction chunks
    KT = S // 128     # 16 key tiles
    TC = S // NQ      # 4 token chunks
    NP = NH // 2      # 8 head pairs

    Exp = mybir.ActivationFunctionType.Exp
    Ident = mybir.ActivationFunctionType.Identity

    with nc.allow_low_precision(reason="bf16/fp32r matmuls; psum stays f32"), \
         tile.TileContext(nc) as tc:
        with tc.tile_pool(name="const", bufs=1) as cst, \
             tc.tile_pool(name="kvp", bufs=1) as kvp, \
             tc.tile_pool(name="qtp", bufs=1) as qtp, \
             tc.tile_pool(name="ap", bufs=1) as ap:

            ones_f32 = cst.tile([128, 128], F32, name="ones_f32")
            ones_sq = cst.tile([128, 128], BF16, name="ones_sq")
            ones_row = cst.tile([1, 128], BF16, name="ones_row")
            nc.vector.memset(ones_f32, 1.0)
            nc.vector.tensor_copy(ones_sq, ones_f32)
            nc.vector.tensor_copy(ones_row, ones_f32[0:1, :])
            bqT = cst.tile([D, NH], F32, name="bqT")
            bkT = cst.tile([D, NKV], F32, name="bkT")
            bvr = cst.tile([1, NKV * D], BF16, name="bvr")
            maskq = cst.tile([D, 4 * D], BF16, name="maskq")
            cosT = cst.tile([D, S], F32R, name="cosT")
            sinT = cst.tile([D, S], F32R, name="sinT")
            nc.sync.dma_start(bqT, bqT_d)
            nc.sync.dma_start(bkT, bkT_d)
            nc.sync.dma_start(bvr, bv_d)
            nc.sync.dma_start(maskq, maskq_d)
            nc.sync.dma_start(cosT, cosT_d)
            nc.sync.dma_start(sinT, sinT_d)

            # persistent K^T [d, t] per kv head; V [t, 2*d] per token tile
            kT_sb = [kvp.tile([128, S], F32R, name=f"kT{g}")
                     for g in range(NKV)]
            v_sb = [kvp.tile([128, NKV * D], BF16, name=f"v{t}")
                    for t in range(KT)]
            # rope'd Q per head pair: [d, 2 heads, 512 q]
            qT2 = [qtp.tile([128, 2, NQ], F32R, name=f"qT2_{p}")
                   for p in range(NP)]
            # both heads' last 128-query window, contiguous for the tail
            qtl = [qtp.tile([128, 256], F32R, name=f"qtl_{p}")
                   for p in range(NP)]
            a_sb = [ap.tile([128, NQ], BF16, name=f"a{h}")
                    for h in range(NH)]

            # strided views: the core's query columns are x' cols 0::4
            def qcols(t):
                return t.rearrange("p (u r) -> p u r", r=4)[:, :, 0]

            with tc.tile_pool(name="xp", bufs=1) as xp:
                x_sb = [xp.tile([128, S], BF16, name=f"x{c}")
                        for c in range(HC)]

                # ------------- P1: K/V projection over all tokens ----------
                with tc.tile_pool(name="p1w", bufs=1) as p1w, \
                     tc.tile_pool(name="p1s", bufs=2) as p1s, \
                     tc.tile_pool(name="p1ps", bufs=1, space="PSUM") as p1ps:
                    wk_sb = [p1w.tile([128, NKV * D], BF16, name=f"wk{c}")
                             for c in range(HC)]
                    wv_sb = [p1w.tile([128, NKV * D], BF16, name=f"wv{c}")
                             for c in range(HC)]
                    for c in range(HC):
                        nc.sync.dma_start(wk_sb[c],
                                          wkT_d[c * 128:(c + 1) * 128, :])
                        nc.sync.dma_start(wv_sb[c],
                                          wvT_d[c * 128:(c + 1) * 128, :])
                    for tch in range(TC):
                        tsl = slice(tch * NQ, (tch + 1) * NQ)
                        for c in range(HC):
                            nc.sync.dma_start(
                                x_sb[c][:, tsl],
                                xT_d[c * 128:(c + 1) * 128, tsl])
                        kps = [p1ps.tile([128, NQ], F32, name=f"kps{g}",
                                         bufs=2) for g in range(NKV)]
                        vps = [p1ps.tile([128, NKV * D], F32, name=f"vps{s_}",
                                         bufs=1) for s_ in range(4)]
                        for c in range(HC):
                            for g in range(NKV):
                                nc.tensor.matmul(
                                    kps[g], wk_sb[c][:, g * D:(g + 1) * D],
                                    x_sb[c][:, tsl],
                                    start=(c == 0), stop=(c == HC - 1))
                            for s_ in range(4):
                                nc.tensor.matmul(
                                    vps[s_],
                                    x_sb[c][:, tch * NQ + s_ * 128:
                                            tch * NQ + (s_ + 1) * 128],
                                    wv_sb[c], start=(c == 0), stop=False)
                        for s_ in range(4):
                            nc.tensor.matmul(vps[s_], ones_row, bvr,
                                             start=False, stop=True)
                            nc.vector.tensor_copy(v_sb[tch * 4 + s_], vps[s_])
                        for g in range(NKV):
                            kb = p1s.tile([128, NQ], F32R, name="kb")
                            nc.scalar.activation(kb, kps[g], Ident,
                                                 bias=bkT[:, g:g + 1])
                            shuf = p1s.tile([128, NQ], F32R, name="shuf")
                            nc.sync.dma_start(shuf[0:64, :], kb[64:128, :])
                            nc.sync.dma_start(shuf[64:128, :], kb[0:64, :])
                            ke = kT_sb[g][:, tsl]
                            nc.vector.tensor_mul(ke, kb, cosT[:, tsl])
                            nc.vector.tensor_mul(shuf, shuf, sinT[:, tsl])
                            nc.vector.tensor_add(ke, ke, shuf)

                # ------------- P2: Q projection + rope ---------------------
                with tc.tile_pool(name="p2w", bufs=3) as p2w, \
                     tc.tile_pool(name="p2s", bufs=2) as p2s, \
                     tc.tile_pool(name="p2ps", bufs=1, space="PSUM") as p2ps:
                    cq = qcols(cosT)
                    sq = qcols(sinT)
                    for h in range(NH):
                        wqh = p2w.tile([128, HID], BF16, name="wqh")
                        nc.sync.dma_start(wqh,
                                          wqP_d[h * 128:(h + 1) * 128, :])
                        qps = p2ps.tile([128, NQ], F32, name="qps", bufs=4)
                        for c in range(HC):
                            nc.tensor.matmul(
                                qps, wqh[:, c * 128:(c + 1) * 128],
                                qcols(x_sb[c]),
                                start=(c == 0), stop=(c == HC - 1))
                        qb = p2s.tile([128, NQ], F32R, name="qb")
                        nc.scalar.activation(qb, qps, Ident,
                                             bias=bqT[:, h:h + 1])
                        shufq = p2s.tile([128, NQ], F32R, name="shufq")
                        nc.sync.dma_start(shufq[0:64, :], qb[64:128, :])
                        nc.sync.dma_start(shufq[64:128, :], qb[0:64, :])
                        qe = qT2[h // 2][:, h % 2, :]
                        nc.vector.tensor_mul(qe, qb, cq)
                        nc.vector.tensor_mul(shufq, shufq, sq)
                        nc.vector.tensor_add(qe, qe, shufq)
                        nc.vector.tensor_copy(
                            qtl[h // 2][:, (h % 2) * 128:(h % 2 + 1) * 128],
                            qe[:, 384:512])

            # ---------------- P3: causal attention -------------------------
            with tc.tile_pool(name="ebp", bufs=3) as ebp, \
                 tc.tile_pool(name="rcp", bufs=2) as rcp, \
                 tc.tile_pool(name="spsp", bufs=1, space="PSUM") as spsp, \
                 tc.tile_pool(name="opsp", bufs=1, space="PSUM") as opsp, \
                 tc.tile_pool(name="stpsp", bufs=1, space="PSUM") as stpsp, \
                 tc.tile_pool(name="tlpsp", bufs=1, space="PSUM") as tlpsp:
                for p in range(NP):
                    g = p // (NP // NKV)
                    acc = []
                    for hh in range(2):
                        ops = opsp.tile([128, NQ], F32, name="ops", bufs=2)
                        stats = stpsp.tile([128, NQ], F32, name="stats",
                                           bufs=2)
                        acc.append((ops, stats))
                        for kt in range(12):
                            off = 128 * (kt // 4)
                            w = slice(off, NQ)
                            sps = spsp.tile([128, NQ], F32, name="sps",
                                            bufs=2)
                            nc.tensor.matmul(
                                sps[:, w],
                                kT_sb[g][:, kt * 128:(kt + 1) * 128],
                                qT2[p][:, hh, w], start=True, stop=True)
                            eb = ebp.tile([128, NQ], BF16, name="eb")
                            nc.scalar.activation(eb[:, w], sps[:, w], Exp,
                                                 scale=float(SM_SCALE))
                            bnd = (kt % 4) * 128
                            nc.vector.tensor_mul(
                                eb[:, off:off + 128], eb[:, off:off + 128],
                                maskq[:, bnd:bnd + 128])
                            nc.tensor.matmul(
                                stats[:, w], ones_sq, eb[:, w],
                                start=(kt == 0), stop=(kt == 11),
                                skip_group_check=True)
                            nc.tensor.matmul(
                                ops[:, w], v_sb[kt][:, g * D:(g + 1) * D],
                                eb[:, w], start=(kt == 0), stop=(kt == 11),
                                skip_group_check=True)
                    # diagonal tail: both heads' last 128-query window
                    tailpv = tlpsp.tile([128, 256], F32, name="tailpv",
                                        bufs=1)
                    tailst = tlpsp.tile([128, 256], F32, name="tailst",
                                        bufs=1)
                    for kt in range(12, 16):
                        sps = spsp.tile([128, NQ], F32, name="sps", bufs=2)
                        nc.tensor.matmul(
                            sps[:, 0:256],
                            kT_sb[g][:, kt * 128:(kt + 1) * 128],
                            qtl[p], start=True, stop=True)
                        eb = ebp.tile([128, NQ], BF16, name="eb")
                        nc.scalar.activation(eb[:, 0:256], sps[:, 0:256],
                                             Exp, scale=float(SM_SCALE))
                        bnd = (kt % 4) * 128
                        nc.vector.tensor_mul(eb[:, 0:128], eb[:, 0:128],
                                             maskq[:, bnd:bnd + 128])
                        nc.vector.tensor_mul(eb[:, 128:256], eb[:, 128:256],
                                             maskq[:, bnd:bnd + 128])
                        nc.tensor.matmul(tailst, ones_sq,
                                         eb[:, 0:256], start=(kt == 12),
                                         stop=(kt == 15),
                                         skip_group_check=True)
                        nc.tensor.matmul(tailpv,
                                         v_sb[kt][:, g * D:(g + 1) * D],
                                         eb[:, 0:256], start=(kt == 12),
                                         stop=(kt == 15),
                                         skip_group_check=True)
                    tailsb = rcp.tile([128, NQ], F32, name="tailsb")
                    nc.vector.tensor_copy(tailsb[:, 0:256], tailpv)
                    nc.vector.tensor_copy(tailsb[:, 256:512], tailst)
                    for hh in range(2):
                        h = 2 * p + hh
                        ops, stats = acc[hh]
                        nc.vector.tensor_add(
                            stats[:, 384:512], stats[:, 384:512],
                            tailsb[:, 256 + 128 * hh:384 + 128 * hh])
                        rec = rcp.tile([128, NQ], F32, name="rec")
                        nc.vector.reciprocal_approx_fast(out=rec, in_=stats)
                        nc.vector.tensor_add(
                            ops[:, 384:512], ops[:, 384:512],
                            tailsb[:, 128 * hh:128 * (hh + 1)])
                        nc.vector.tensor_mul(a_sb[h], ops, rec)

            # ---------------- P4: o_proj -----------------------------------
            with tc.tile_pool(name="wop", bufs=1) as wop, \
                 tc.tile_pool(name="wos", bufs=3) as wos, \
                 tc.tile_pool(name="wops", bufs=1, space="PSUM") as wops:
                for ec in range(4):
                    wo_t = [wop.tile([128, NQ], BF16, name=f"wo{h}",
                                     bufs=2) for h in range(NH)]
                    for h in range(NH):
                        nc.sync.dma_start(
                            wo_t[h], woT_d[h * 128:(h + 1) * 128,
                                           ec * NQ:(ec + 1) * NQ])
                    for qs_ in range(4):
                        opo = wops.tile([128, NQ], F32, name="opo", bufs=3)
                        for h in range(NH):
                            nc.tensor.matmul(
                                opo, a_sb[h][:, qs_ * 128:(qs_ + 1) * 128],
                                wo_t[h], start=(h == 0), stop=(h == NH - 1))
                        osb = wos.tile([128, NQ], F32, name="osb")
                        nc.vector.tensor_copy(osb, opo)
                        nc.sync.dma_start(
                            out_d[qs_ * 128:(qs_ + 1) * 128,
                                  ec * NQ:(ec + 1) * NQ], osb)
    return nc


_MROPE_SECTION = [16, 24, 24]
_STREAM_IDX = np.concatenate(
    [np.full(n, i % 3, np.int64)
     for i, n in enumerate(_MROPE_SECTION * 2)])  # [128]


def _causal_mask_ok(mask):
    """True iff attention_mask is exactly the canonical causal mask."""
    m = np.asarray(mask, np.float32)
    if m.shape != (B, 1, S, S):
        return False
    tri = np.tril(np.ones((S, S), bool))
    canon = np.where(tri, np.float32(0.0), np.float32(NEG))
    return all(np.array_equal(m[b, 0], canon) for b in range(B))


def _host_prep_causal(hidden_states, cos, sin, Wq, bq, Wk, bk, Wv, bv, Wo):
    f = np.float32
    hs = np.asarray(hidden_states, f)
    cos = np.asarray(cos, f)
    sin = np.asarray(sin, f)
    ar = np.arange(D)

    wqP = _bf16(np.asarray(Wq, f).reshape(NH, D, HC_G, 128)
                .transpose(0, 3, 2, 1).reshape(HID, HID))
    shared = {
        "wqP": wqP,
        "wkT": _bf16(np.asarray(Wk, f).T),
        "wvT": _bf16(np.asarray(Wv, f).T),
        "woT": _bf16(np.asarray(Wo, f).T),
        "bqT": np.ascontiguousarray(np.asarray(bq, f).reshape(NH, D).T),
        "bkT": np.ascontiguousarray(np.asarray(bk, f).reshape(NKV, D).T),
        "bv": _bf16(np.asarray(bv, f).reshape(1, NKV * D)),
    }

    base = []
    for b in range(B):
        xT = hs[b].T                              # [HID, S]
        cosT = cos[_STREAM_IDX, b, :, ar]         # [128, S]
        sinT = sin[_STREAM_IDX, b, :, ar].copy()
        sinT[0:64, :] *= -1.0
        base.append((xT, cosT, sinT))

    i = np.arange(S)
    u, r = i // 4, i % 4
    t512 = np.arange(D)
    mm = np.arange(D)[:, None]                    # k within band tile

    in_maps = []
    for c in range(N_CORES):
        b, c4 = divmod(c, N_CORES // B)
        xT, cosT, sinT = base[b]
        gperm = 4 * u + (r + c4) % 4              # x' col -> global col
        m = dict(shared)
        m["xT"] = _bf16(xT[:, gperm])
        m["cosT"] = _round_fp32r(cosT[:, gperm])
        m["sinT"] = _round_fp32r(sinT[:, gperm])
        # maskq[k, band*128 + t]: key vs query within the diagonal band
        bands = []
        for band in range(4):
            mloc = 128 * band + mm                # in-band x' index
            kglob = 4 * (mloc // 4) + ((mloc % 4) + c4) % 4
            bands.append((kglob <= 4 * t512[None, :] + c4))
        m["maskq"] = _bf16(np.concatenate(bands, axis=1).astype(f))
        in_maps.append(m)
    return in_maps


HC_G = HID // 128


# ---------------------------------------------------------------------------
# dense fallback (original kernel, arbitrary additive mask via exp(mask))
# ---------------------------------------------------------------------------

def _build_nc(mm="f32r"):
    import contextlib
    import concourse.bass as bass  # noqa: F401
    import concourse.tile as tile
    from concourse import bacc, mybir

    F32 = mybir.dt.float32
    MMDT = mybir.dt.float32r if mm == "f32r" else F32

    nc = bacc.Bacc(target_bir_lowering=False, debug=False)

    def param(name, shape, dt=MMDT):
        return nc.declare_dram_parameter(name, list(shape), dt,
                                         isOutput=False)[:]

    xT = param("xT", [HID, S])
    wqT = param("wqT", [HID, HID])
    wkT = param("wkT", [HID, NKV * D])
    wvT = param("wvT", [HID, NKV * D])
    woT = param("woT", [HID, HID])
    bqT_d = param("bqT", [D, NH], F32)
    bkT_d = param("bkT", [D, NKV], F32)
    bv_d = param("bv", [1, NKV * D])
    cosT_d = param("cosT", [D, S])
    sinT_d = param("sinT", [D, S])
    cq_d = param("cosTq", [D, NQ])
    sq_d = param("sinTq", [D, NQ])
    maskT_d = param("maskT", [S, NQ])     # exp(mask).T, fp32r-rounded
    out_d = nc.declare_dram_parameter("out", [NQ, HID], F32, isOutput=True)[:]

    HC = HID // 128   # 16 contraction chunks
    KT = S // 128     # 16 key tiles
    KT2 = KT // 2     # 8 key tile-pairs
    TC = S // NQ      # 4 token chunks (for K/V proj)
    QS = NQ // 128    # 4 query sub-tiles

    Exp = mybir.ActivationFunctionType.Exp
    Ident = mybir.ActivationFunctionType.Identity

    lp = (nc.allow_low_precision(reason="fp32r matmul operands; psum stays f32")
          if mm == "f32r" else contextlib.nullcontext())
    with lp, tile.TileContext(nc) as tc:
        with tc.tile_pool(name="const", bufs=1) as cst, \
             tc.tile_pool(name="maskp", bufs=1) as maskp, \
             tc.tile_pool(name="kvp", bufs=1) as kvp:

            ones_row = cst.tile([1, 128], MMDT, name="ones_row")
            ones_sq = cst.tile([128, 128], MMDT, name="ones_sq")
            ones_f32 = cst.tile([128, 128], F32, name="ones_f32")
            nc.vector.memset(ones_f32, 1.0)
            nc.vector.tensor_copy(ones_row, ones_f32[0:1, :])
            nc.vector.tensor_copy(ones_sq, ones_f32)
            bqT = cst.tile([D, NH], F32, name="bqT")
            bkT = cst.tile([D, NKV], F32, name="bkT")
            bvr = cst.tile([1, NKV * D], MMDT, name="bvr")
            nc.sync.dma_start(bqT, bqT_d)
            nc.sync.dma_start(bkT, bkT_d)
            nc.sync.dma_start(bvr, bv_d)

            # exp(mask) tiles [128 k, 2 kt, 512 q], resident through attention
            mask_sb = [maskp.tile([128, 2, NQ], MMDT, name=f"mask{kt}")
                       for kt in range(KT2)]

            # token chunk 0 of xT = this core's query columns (host permutes
            # chunks); kept resident for the Q projection
            xq_sb = [kvp.tile([128, NQ], MMDT, name=f"xq{c}")
                     for c in range(HC)]
            # persistent K^T [d, t] per kv head; V [t, d] per token tile
            kT_sb = [kvp.tile([128, S], MMDT, name=f"kT{g}")
                     for g in range(NKV)]
            v_sb = [kvp.tile([128, NKV * D], MMDT, name=f"v{t}")
                    for t in range(KT)]

            # ---------------- P1a: K/V projection over all tokens ----------
            with tc.tile_pool(name="p1", bufs=1) as p1, \
                 tc.tile_pool(name="p1s", bufs=3) as p1s, \
                 tc.tile_pool(name="p1ps", bufs=1, space="PSUM") as p1ps:
                wk_sb = [p1.tile([128, NKV * D], MMDT, name=f"wk{c}")
                         for c in range(HC)]
                wv_sb = [p1.tile([128, NKV * D], MMDT, name=f"wv{c}")
                         for c in range(HC)]

                for tch in range(TC):
                    tsl = slice(tch * NQ, (tch + 1) * NQ)
                    kps = [p1ps.tile([128, NQ], F32, name=f"kps{g}", bufs=2)
                           for g in range(NKV)]
                    vps = [p1ps.tile([128, NKV * D], F32, name=f"vps{s_}",
                                     bufs=1) for s_ in range(4)]
                    for c in range(HC):
                        if tch == 0:
                            nc.sync.dma_start(wk_sb[c],
                                              wkT[c * 128:(c + 1) * 128, :])
                            nc.sync.dma_start(wv_sb[c],
                                              wvT[c * 128:(c + 1) * 128, :])
                            xt = xq_sb[c]
                        else:
                            xt = p1s.tile([128, NQ], MMDT, name="xt",
                                          bufs=8)
                        nc.sync.dma_start(xt, xT[c * 128:(c + 1) * 128, tsl])
                        for g in range(NKV):
                            nc.tensor.matmul(
                                kps[g], wk_sb[c][:, g * D:(g + 1) * D],
                                xt, start=(c == 0), stop=(c == HC - 1))
                        for s_ in range(4):
                            nc.tensor.matmul(
                                vps[s_], xt[:, s_ * 128:(s_ + 1) * 128],
                                wv_sb[c], start=(c == 0), stop=False)
                    # V bias via K=1 ones matmul, then evacuate
                    for s_ in range(4):
                        nc.tensor.matmul(vps[s_], ones_row, bvr,
                                         start=False, stop=True)
                        nc.vector.tensor_copy(v_sb[tch * 4 + s_], vps[s_])
                    # K bias + rope -> kT_sb
                    csb = p1s.tile([128, NQ], MMDT, name="csb")
                    ssb = p1s.tile([128, NQ], MMDT, name="ssb")
                    nc.sync.dma_start(csb, cosT_d[:, tsl])
                    nc.sync.dma_start(ssb, sinT_d[:, tsl])
                    for g in range(NKV):
                        kb = p1s.tile([128, NQ], MMDT, name="kb")
                        nc.scalar.activation(kb, kps[g], Ident,
                                             bias=bkT[:, g:g + 1])
                        ke = kT_sb[g][:, tsl]
                        shuf = p1s.tile([128, NQ], MMDT, name="shuf")
                        nc.sync.dma_start(shuf[0:64, :], kb[64:128, :])
                        nc.sync.dma_start(shuf[64:128, :], kb[0:64, :])
                        nc.vector.tensor_mul(ke, kb, csb)
                        nc.vector.tensor_mul(shuf, shuf, ssb)
                        nc.vector.tensor_add(ke, ke, shuf)

            # -------- P1b + P2: Q proj interleaved with attention ----------
            with tc.tile_pool(name="ap", bufs=1) as ap:
                a_sb = [ap.tile([128, NQ], MMDT, name=f"a{h}")
                        for h in range(NH)]
                with tc.tile_pool(name="p2", bufs=1) as p2, \
                     tc.tile_pool(name="p2s", bufs=2) as p2s, \
                     tc.tile_pool(name="p2w", bufs=10) as p2w, \
                     tc.tile_pool(name="qtp", bufs=2) as qtp, \
                     tc.tile_pool(name="att", bufs=5) as att, \
                     tc.tile_pool(name="atts", bufs=2) as atts:
                    cq = p2.tile([D, NQ], MMDT, name="cq")
                    sq = p2.tile([D, NQ], MMDT, name="sq")
                    nc.sync.dma_start(cq, cq_d)
                    nc.sync.dma_start(sq, sq_d)

                    for hg in range(4):
                        qT_sb = {}
                        with tc.tile_pool(name=f"qps{hg}", bufs=1,
                                          space="PSUM") as p2ps:
                            qps = [p2ps.tile([128, NQ], F32, name=f"qps{j}",
                                             bufs=1) for j in range(4)]
                            for c in range(HC):
                                wq = p2w.tile([128, NQ], MMDT, name="wq")
                                nc.sync.dma_start(
                                    wq, wqT[c * 128:(c + 1) * 128,
                                            hg * NQ:(hg + 1) * NQ])
                                for j in range(4):
                                    nc.tensor.matmul(
                                        qps[j], wq[:, j * 128:(j + 1) * 128],
                                        xq_sb[c], start=(c == 0),
                                        stop=(c == HC - 1))
                            for j in range(4):
                                h = hg * 4 + j
                                qT_sb[h] = qtp.tile([128, NQ], MMDT,
                                                    name=f"qT{j}")
                                qb = p2s.tile([128, NQ], MMDT, name="qb")
                                nc.scalar.activation(qb, qps[j], Ident,
                                                     bias=bqT[:, h:h + 1])
                                qe = qT_sb[h]
                                shufq = p2s.tile([128, NQ], MMDT,
                                                 name="shufq")
                                nc.sync.dma_start(shufq[0:64, :],
                                                  qb[64:128, :])
                                nc.sync.dma_start(shufq[64:128, :],
                                                  qb[0:64, :])
                                nc.vector.tensor_mul(qe, qb, cq)
                                nc.vector.tensor_mul(shufq, shufq, sq)
                                nc.vector.tensor_add(qe, qe, shufq)

                        if hg == 0:
                            for kt2 in range(KT2):
                                nc.sync.dma_start(
                                    mask_sb[kt2],
                                    maskT_d[256 * kt2:256 * (kt2 + 1),
                                            :].rearrange(
                                        "(a p) q -> p a q", a=2))
                        with tc.tile_pool(name=f"attps{hg}", bufs=1,
                                          space="PSUM") as attps:
                            for h in range(hg * 4, hg * 4 + 4):
                                g = h // (NH // NKV)
                                ops = attps.tile([128, NQ], F32, name="ops",
                                                 bufs=1)
                                stats = attps.tile([128, NQ], F32,
                                                   name="stats", bufs=1)
                                for kt2 in range(KT2):
                                    sps = attps.tile([128, 2, NQ], F32,
                                                     name="sps", bufs=3)
                                    ebuf = att.tile([128, 2, NQ], MMDT,
                                                    name="ebuf")
                                    for j2 in range(2):
                                        kt = 2 * kt2 + j2
                                        nc.tensor.matmul(
                                            sps[:, j2, :],
                                            kT_sb[g][:, kt * 128:
                                                     (kt + 1) * 128],
                                            qT_sb[h], start=True, stop=True)
                                    nc.scalar.activation(
                                        ebuf.rearrange("p a b -> p (a b)"),
                                        sps.rearrange("p a b -> p (a b)"),
                                        Exp, scale=float(SM_SCALE))
                                    nc.vector.tensor_mul(
                                        ebuf.rearrange("p a b -> p (a b)"),
                                        ebuf.rearrange("p a b -> p (a b)"),
                                        mask_sb[kt2].rearrange(
                                            "p a b -> p (a b)"))
                                    for j2 in range(2):
                                        kt = 2 * kt2 + j2
                                        nc.tensor.matmul(
                                            stats, ones_sq, ebuf[:, j2, :],
                                            start=(kt == 0),
                                            stop=(kt == KT - 1))
                                        nc.tensor.matmul(
                                            ops,
                                            v_sb[kt][:, g * D:(g + 1) * D],
                                            ebuf[:, j2, :],
                                            start=(kt == 0),
                                            stop=(kt == KT - 1))
                                recip = atts.tile([128, NQ], F32,
                                                  name="recip")
                                nc.vector.reciprocal_approx_fast(
                                    out=recip, in_=stats)
                                nc.vector.tensor_mul(a_sb[h], ops, recip)

                # ------------- P3: o_proj ------------------------------
                with tc.tile_pool(name="wop", bufs=1) as wop, \
                     tc.tile_pool(name="wos", bufs=3) as wos, \
                     tc.tile_pool(name="wops", bufs=1, space="PSUM") as wops:
                    for ec in range(4):
                        wo_t = [wop.tile([128, NQ], MMDT, name=f"wo{h}",
                                         bufs=2) for h in range(NH)]
                        for h in range(NH):
                            nc.sync.dma_start(
                                wo_t[h], woT[h * 128:(h + 1) * 128,
                                             ec * NQ:(ec + 1) * NQ])
                        for qs_ in range(QS):
                            opo = wops.tile([128, NQ], F32, name="opo",
                                            bufs=3)
                            for h in range(NH):
                                nc.tensor.matmul(
                                    opo,
                                    a_sb[h][:, qs_ * 128:(qs_ + 1) * 128],
                                    wo_t[h], start=(h == 0),
                                    stop=(h == NH - 1))
                            osb = wos.tile([128, NQ], F32, name="osb")
                            nc.vector.tensor_copy(osb, opo)
                            nc.sync.dma_start(
                                out_d[qs_ * 128:(qs_ + 1) * 128,
                                      ec * NQ:(ec + 1) * NQ], osb)
    return nc


def get_nc(key):
    if key not in _BUILD_CACHE:
        if key == "causal":
            nc = _build_causal()
        else:
            nc = _build_nc(key)
        nc.finalize()
        _BUILD_CACHE[key] = nc
    return _BUILD_CACHE[key]


def _host_prep(hidden_states, cos, sin, attention_mask, Wq, bq, Wk, bk, Wv,
               bv, Wo, mm="f32r"):
    f = np.float32
    if mm == "f32r":
        rnd = _round_fp32r
    else:
        def rnd(a):
            return np.ascontiguousarray(a, f)
    hs = np.asarray(hidden_states, f)
    cos = np.asarray(cos, f)
    sin = np.asarray(sin, f)
    mask = np.asarray(attention_mask, f)
    ar = np.arange(D)

    shared = {
        "wqT": rnd(np.asarray(Wq, f).T),
        "wkT": rnd(np.asarray(Wk, f).T),
        "wvT": rnd(np.asarray(Wv, f).T),
        "woT": rnd(np.asarray(Wo, f).T),
        "bqT": np.ascontiguousarray(np.asarray(bq, f).reshape(NH, D).T),
        "bkT": np.ascontiguousarray(np.asarray(bk, f).reshape(NKV, D).T),
        "bv": rnd(np.asarray(bv, f).reshape(1, NKV * D)),
    }

    per_batch = []
    for b in range(B):
        xT = rnd(hs[b].T)
        cosT = rnd(cos[_STREAM_IDX, b, :, ar])  # [128, S]
        sinT = rnd(sin[_STREAM_IDX, b, :, ar])
        sinT[0:64, :] *= -1.0   # rotate_half sign folded into sin
        maskT = rnd(np.exp(mask[b, 0].T.astype(np.float64)
                           ).astype(np.float32))
        per_batch.append((xT, cosT, sinT, maskT))

    in_maps = []
    for c in range(N_CORES):
        b, qc = divmod(c, N_CORES // B)
        xT, cosT, sinT, maskT = per_batch[b]
        qsl = slice(qc * NQ, (qc + 1) * NQ)
        order = [qc] + [o for o in range(N_CORES // B) if o != qc]
        tperm = np.concatenate([np.arange(o * NQ, (o + 1) * NQ)
                                for o in order])
        m = dict(shared)
        m["xT"] = np.ascontiguousarray(xT[:, tperm])
        m["cosT"] = np.ascontiguousarray(cosT[:, tperm])
        m["sinT"] = np.ascontiguousarray(sinT[:, tperm])
        m["maskT"] = np.ascontiguousarray(maskT[tperm][:, qsl])
        m["cosTq"] = np.ascontiguousarray(cosT[:, qsl])
        m["sinTq"] = np.ascontiguousarray(sinT[:, qsl])
        in_maps.append(m)
    return in_maps


def kernel(hidden_states, cos, sin, attention_mask, Wq, bq, Wk, bk, Wv, bv,
           Wo, _trace=False, _mm="causal"):
    from concourse.bass_utils import run_bass_kernel_spmd

    if _mm == "causal" and not _causal_mask_ok(attention_mask):
        _mm = "f32r"

    if _mm == "causal":
        in_maps = _host_prep_causal(hidden_states, cos, sin, Wq, bq, Wk, bk,
                                    Wv, bv, Wo)
        nc = get_nc("causal")
        res = run_bass_kernel_spmd(nc, in_maps, list(range(N_CORES)),
                                   trace=_trace)
        out = np.empty((B, S, HID), np.float32)
        for c in range(N_CORES):
            b, c4 = divmod(c, N_CORES // B)
            out[b, c4::4, :] = res.results[c]["out"]
        kernel._last_results = res
        return out

    in_maps = _host_prep(hidden_states, cos, sin, attention_mask, Wq, bq, Wk,
                         bk, Wv, bv, Wo, mm=_mm)
    nc = get_nc(_mm)
    res = run_bass_kernel_spmd(nc, in_maps, list(range(N_CORES)),
                               trace=_trace)
    out = np.empty((B, S, HID), np.float32)
    for c in range(N_CORES):
        b, qc = divmod(c, N_CORES // B)
        out[b, qc * NQ:(qc + 1) * NQ, :] = res.results[c]["out"]
    kernel._last_results = res
    return out


# revision 18
# speedup vs baseline: 1.5098x; 1.1952x over previous
"""Qwen2.5-VL attention (mrope + GQA + causal mask + o_proj) on 8 Trainium2
NeuronCores.

Fast path (causal mask detected): batch x interleaved-query sharding.
Core c handles batch b = c//4 and the stride-4 query comb {c4, c4+4, ...}
(c4 = c%4).  Host permutes tokens within every aligned group of 4 so the
core's queries sit at on-device columns 0::4; key tiles of 128 then cover
the same global token sets on every core, giving an SPMD-uniform causal
structure: query sub-tile j (128 rows, global span [512j, 512j+512)) only
attends key tiles kt < 4(j+1).

Attention loop per head: 12 "main" key tiles with shrinking query windows
(512/384/256 wide, always >=256 so fp32r/bf16 matmuls stay 1 cycle/row)
plus a 4-tile diagonal "tail" computed jointly for a head PAIR (the two
heads' last 128-query windows concatenated to free dim 256).  Only the
single diagonal key tile per iteration needs the 0/1 mask multiply; fully
masked tiles are skipped, fully visible tiles skip the multiply.

Softmax denominators via ones[128,128] matmuls accumulated region-wise in
PSUM alongside PV (start/stop flags per region, skip_group_check).

dtypes: projections stream bf16 (x, Wq, Wk, Wv, Wo); rope + scores run in
fp32r (cos/sin, qT, kT tiles); exp output / V / attn weights bf16; all
PSUM accumulation f32.  Measured rel err ~6e-3 vs the f32 reference.

Fallback (non-causal mask): the original dense kernel (exp(mask)
multiplicative masking over all tiles, fp32r everywhere).
"""

import sys

for _p in ("/opt/trn_rl_repo", "/root/.axon_site/_ro/trn_rl_repo"):
    if _p not in sys.path:
        sys.path.insert(0, _p)

import numpy as np

B = 2
S = 2048
HID = 2048
NH = 16
NKV = 2
D = 128
NQ = 512          # query rows per core
N_CORES = 8
SM_SCALE = 1.0 / np.sqrt(np.float32(D))
NEG = -3.4028235e38

_BUILD_CACHE = {}


def _round_fp32r(a):
    """Round-to-nearest-even to 12 explicit mantissa bits (fp32r)."""
    u = np.ascontiguousarray(a, np.float32).view(np.uint32)
    low = u & np.uint32(0xFFF)
    up = (u & np.uint32(0xFFFFF000)) + np.uint32(0x1000)
    half = low == np.uint32(0x800)
    rnd = np.where(low > 0x800, up,
                   np.where(half & ((u & np.uint32(0x1000)) != 0), up,
                            u & np.uint32(0xFFFFF000)))
    expmask = (u & np.uint32(0x7F800000)) == np.uint32(0x7F800000)
    rnd = np.where(expmask, u, rnd)
    return rnd.view(np.float32)


def _bf16(a):
    import ml_dtypes
    return np.asarray(a, np.float32).astype(ml_dtypes.bfloat16)


# ---------------------------------------------------------------------------
# causal fast path
# ---------------------------------------------------------------------------

def _build_causal():
    import concourse.bass as bass  # noqa: F401 (env init)
    import concourse.tile as tile
    from concourse import bacc, mybir

    F32 = mybir.dt.float32
    F32R = mybir.dt.float32r
    BF16 = mybir.dt.bfloat16

    nc = bacc.Bacc(target_bir_lowering=False, debug=False)

    def param(name, shape, dt):
        return nc.declare_dram_parameter(name, list(shape), dt,
                                         isOutput=False)[:]

    xT_d = param("xT", [HID, S], BF16)          # permuted token order
    xq_d = param("xq", [HID, NQ], BF16)         # this core's query columns
    wqP_d = param("wqP", [HID, HID], BF16)      # [h*128+k, c*128+m]
    wkvT_d = param("wkvT", [HID, 2 * NKV * D], BF16)   # [Wk.T | Wv.T]
    woT_d = param("woT", [HID, HID], BF16)
    bqT_d = param("bqT", [D, NH], F32)
    bkT_d = param("bkT", [D, NKV], F32)
    bv_d = param("bv", [1, NKV * D], BF16)
    cosT_d = param("cosT", [D, S], F32R)        # permuted, mrope-merged
    sinT_d = param("sinT", [D, S], F32R)        # top half sign-flipped
    cosq_d = param("cosq", [D, NQ], F32R)       # query columns
    sinq_d = param("sinq", [D, NQ], F32R)
    maskq_d = param("maskq", [D, 4 * D], BF16)  # diag band 0/1, j-invariant
    out_d = nc.declare_dram_parameter("out", [NQ, HID], F32, isOutput=True)[:]

    HC = HID // 128   # 16 contraction chunks
    KT = S // 128     # 16 key tiles
    TC = S // NQ      # 4 token chunks
    NP = NH // 2      # 8 head pairs

    Exp = mybir.ActivationFunctionType.Exp
    Ident = mybir.ActivationFunctionType.Identity

    with nc.allow_low_precision(reason="bf16/fp32r matmuls; psum stays f32"), \
         tile.TileContext(nc) as tc:
        with tc.tile_pool(name="const", bufs=1) as cst, \
             tc.tile_pool(name="kvp", bufs=1) as kvp, \
             tc.tile_pool(name="qtp", bufs=1) as qtp, \
             tc.tile_pool(name="ap", bufs=1) as ap:

            ones_f32 = cst.tile([128, 128], F32, name="ones_f32")
            ones_sq = cst.tile([128, 128], BF16, name="ones_sq")
            ones_row = cst.tile([1, 128], BF16, name="ones_row")
            nc.vector.memset(ones_f32, 1.0)
            nc.vector.tensor_copy(ones_sq, ones_f32)
            nc.vector.tensor_copy(ones_row, ones_f32[0:1, :])
            bqT = cst.tile([D, NH], F32, name="bqT")
            bkT = cst.tile([D, NKV], F32, name="bkT")
            bvr = cst.tile([1, NKV * D], BF16, name="bvr")
            maskq = cst.tile([D, 4 * D], BF16, name="maskq")
            cosT = cst.tile([D, S], F32R, name="cosT")
            sinT = cst.tile([D, S], F32R, name="sinT")
            cosq = cst.tile([D, NQ], F32R, name="cosq")
            sinq = cst.tile([D, NQ], F32R, name="sinq")
            nc.sync.dma_start(bqT, bqT_d)
            nc.sync.dma_start(bkT, bkT_d)
            nc.sync.dma_start(bvr, bv_d)
            nc.sync.dma_start(maskq, maskq_d)
            nc.sync.dma_start(cosT, cosT_d)
            nc.sync.dma_start(sinT, sinT_d)
            nc.sync.dma_start(cosq, cosq_d)
            nc.sync.dma_start(sinq, sinq_d)

            # persistent K^T [d, t] per kv head; V [t, 2*d] per token tile
            kT_sb = [kvp.tile([128, S], F32R, name=f"kT{g}")
                     for g in range(NKV)]
            v_sb = [kvp.tile([128, NKV * D], BF16, name=f"v{t}")
                    for t in range(KT)]
            # rope'd Q per head pair: [d, 2 heads, 512 q]
            qT2 = [qtp.tile([128, 2, NQ], F32R, name=f"qT2_{p}")
                   for p in range(NP)]
            # both heads' last 128-query window, contiguous for the tail
            qtl = [qtp.tile([128, 256], F32R, name=f"qtl_{p}")
                   for p in range(NP)]
            a_sb = [ap.tile([128, NQ], BF16, name=f"a{h}")
                    for h in range(NH)]

            # strided views: the core's query columns are x' cols 0::4
            def qcols(t):
                return t.rearrange("p (u r) -> p u r", r=4)[:, :, 0]

            with tc.tile_pool(name="xp", bufs=1) as xp:
                x_sb = [xp.tile([128, S], BF16, name=f"x{c}")
                        for c in range(HC)]
                xq_sb = [xp.tile([128, NQ], BF16, name=f"xq{c}")
                         for c in range(HC)]

                # ------------- P1: K/V projection over all tokens ----------
                with tc.tile_pool(name="p1w", bufs=1) as p1w, \
                     tc.tile_pool(name="p1s", bufs=2) as p1s, \
                     tc.tile_pool(name="p1ps", bufs=1, space="PSUM") as p1ps:
                    wkv_sb = [p1w.tile([128, 2 * NKV * D], BF16,
                                       name=f"wkv{c}") for c in range(HC)]
                    for c in range(HC):
                        nc.gpsimd.dma_start(
                            x_sb[c], xT_d[c * 128:(c + 1) * 128, :])
                        nc.gpsimd.dma_start(
                            wkv_sb[c], wkvT_d[c * 128:(c + 1) * 128, :])
                    for c in range(HC):
                        nc.gpsimd.dma_start(
                            xq_sb[c], xq_d[c * 128:(c + 1) * 128, :])
                    for tch in range(TC):
                        tsl = slice(tch * NQ, (tch + 1) * NQ)
                        kps = [p1ps.tile([128, NQ], F32, name=f"kps{g}",
                                         bufs=2) for g in range(NKV)]
                        vps = [p1ps.tile([128, NKV * D], F32, name=f"vps{s_}",
                                         bufs=1) for s_ in range(4)]
                        for c in range(HC):
                            for g in range(NKV):
                                nc.tensor.matmul(
                                    kps[g], wkv_sb[c][:, g * D:(g + 1) * D],
                                    x_sb[c][:, tsl],
                                    start=(c == 0), stop=(c == HC - 1))
                            for s_ in range(4):
                                nc.tensor.matmul(
                                    vps[s_],
                                    x_sb[c][:, tch * NQ + s_ * 128:
                                            tch * NQ + (s_ + 1) * 128],
                                    wkv_sb[c][:, NKV * D:2 * NKV * D],
                                    start=(c == 0), stop=False)
                        for s_ in range(4):
                            nc.tensor.matmul(vps[s_], ones_row, bvr,
                                             start=False, stop=True)
                            nc.vector.tensor_copy(v_sb[tch * 4 + s_], vps[s_])
                        for g in range(NKV):
                            kb = p1s.tile([128, NQ], F32R, name="kb")
                            nc.scalar.activation(kb, kps[g], Ident,
                                                 bias=bkT[:, g:g + 1])
                            shuf = p1s.tile([128, NQ], F32R, name="shuf")
                            nc.sync.dma_start(shuf[0:64, :], kb[64:128, :])
                            nc.sync.dma_start(shuf[64:128, :], kb[0:64, :])
                            ke = kT_sb[g][:, tsl]
                            nc.vector.tensor_mul(ke, kb, cosT[:, tsl])
                            nc.vector.tensor_mul(shuf, shuf, sinT[:, tsl])
                            nc.vector.tensor_add(ke, ke, shuf)

                # ------------- P2: Q projection + rope ---------------------
                with tc.tile_pool(name="p2w", bufs=3) as p2w, \
                     tc.tile_pool(name="p2s", bufs=2) as p2s, \
                     tc.tile_pool(name="p2ps", bufs=1, space="PSUM") as p2ps:
                    for h in range(NH):
                        wqh = p2w.tile([128, HID], BF16, name="wqh")
                        nc.gpsimd.dma_start(wqh,
                                            wqP_d[h * 128:(h + 1) * 128, :])
                        qps = p2ps.tile([128, NQ], F32, name="qps", bufs=4)
                        for c in range(HC):
                            nc.tensor.matmul(
                                qps, wqh[:, c * 128:(c + 1) * 128],
                                xq_sb[c],
                                start=(c == 0), stop=(c == HC - 1))
                        qb = p2s.tile([128, NQ], F32R, name="qb")
                        nc.scalar.activation(qb, qps, Ident,
                                             bias=bqT[:, h:h + 1])
                        shufq = p2s.tile([128, NQ], F32R, name="shufq")
                        nc.sync.dma_start(shufq[0:64, :], qb[64:128, :])
                        nc.sync.dma_start(shufq[64:128, :], qb[0:64, :])
                        qe = qT2[h // 2][:, h % 2, :]
                        nc.vector.tensor_mul(qe, qb, cosq)
                        nc.vector.tensor_mul(shufq, shufq, sinq)
                        nc.vector.tensor_add(qe, qe, shufq)
                        nc.vector.tensor_copy(
                            qtl[h // 2][:, (h % 2) * 128:(h % 2 + 1) * 128],
                            qe[:, 384:512])

            # ---------------- P3: causal attention -------------------------
            # (o_proj weights prefetch during attention via the idle Pool
            #  DMA queue; the wop pool outlives P3 into P4)
            wop_cm = tc.tile_pool(name="wop", bufs=1)
            wop = wop_cm.__enter__()
            wo_t = [[wop.tile([128, NQ], BF16, name=f"wo{ec}_{h}")
                     for h in range(NH)] for ec in range(4)]
            for ec in range(4):
                for h in range(NH):
                    nc.gpsimd.dma_start(
                        wo_t[ec][h], woT_d[h * 128:(h + 1) * 128,
                                           ec * NQ:(ec + 1) * NQ])
            with tc.tile_pool(name="ebp", bufs=3) as ebp, \
                 tc.tile_pool(name="rcp", bufs=2) as rcp, \
                 tc.tile_pool(name="spsp", bufs=1, space="PSUM") as spsp, \
                 tc.tile_pool(name="opsp", bufs=1, space="PSUM") as opsp, \
                 tc.tile_pool(name="stpsp", bufs=1, space="PSUM") as stpsp, \
                 tc.tile_pool(name="tlpsp", bufs=1, space="PSUM") as tlpsp:
                for p in range(NP):
                    g = p // (NP // NKV)
                    acc = []
                    for hh in range(2):
                        ops = opsp.tile([128, NQ], F32, name="ops", bufs=2)
                        stats = stpsp.tile([128, NQ], F32, name="stats",
                                           bufs=2)
                        acc.append((ops, stats))
                        for kt in range(12):
                            off = 128 * (kt // 4)
                            w = slice(off, NQ)
                            sps = spsp.tile([128, NQ], F32, name="sps",
                                            bufs=2)
                            nc.tensor.matmul(
                                sps[:, w],
                                kT_sb[g][:, kt * 128:(kt + 1) * 128],
                                qT2[p][:, hh, w], start=True, stop=True)
                            eb = ebp.tile([128, NQ], BF16, name="eb")
                            nc.scalar.activation(eb[:, w], sps[:, w], Exp,
                                                 scale=float(SM_SCALE))
                            bnd = (kt % 4) * 128
                            nc.vector.tensor_mul(
                                eb[:, off:off + 128], eb[:, off:off + 128],
                                maskq[:, bnd:bnd + 128])
                            nc.tensor.matmul(
                                stats[:, w], ones_sq, eb[:, w],
                                start=(kt == 0), stop=(kt == 11),
                                skip_group_check=True)
                            nc.tensor.matmul(
                                ops[:, w], v_sb[kt][:, g * D:(g + 1) * D],
                                eb[:, w], start=(kt == 0), stop=(kt == 11),
                                skip_group_check=True)
                    # diagonal tail: both heads' last 128-query window
                    tailpv = tlpsp.tile([128, 256], F32, name="tailpv",
                                        bufs=1)
                    tailst = tlpsp.tile([128, 256], F32, name="tailst",
                                        bufs=1)
                    for kt in range(12, 16):
                        sps = spsp.tile([128, NQ], F32, name="sps", bufs=2)
                        nc.tensor.matmul(
                            sps[:, 0:256],
                            kT_sb[g][:, kt * 128:(kt + 1) * 128],
                            qtl[p], start=True, stop=True)
                        eb = ebp.tile([128, NQ], BF16, name="eb")
                        nc.scalar.activation(eb[:, 0:256], sps[:, 0:256],
                                             Exp, scale=float(SM_SCALE))
                        bnd = (kt % 4) * 128
                        nc.vector.tensor_mul(eb[:, 0:128], eb[:, 0:128],
                                             maskq[:, bnd:bnd + 128])
                        nc.vector.tensor_mul(eb[:, 128:256], eb[:, 128:256],
                                             maskq[:, bnd:bnd + 128])
                        nc.tensor.matmul(tailst, ones_sq,
                                         eb[:, 0:256], start=(kt == 12),
                                         stop=(kt == 15),
                                         skip_group_check=True)
                        nc.tensor.matmul(tailpv,
                                         v_sb[kt][:, g * D:(g + 1) * D],
                                         eb[:, 0:256], start=(kt == 12),
                                         stop=(kt == 15),
                                         skip_group_check=True)
                    tailsb = rcp.tile([128, NQ], F32, name="tailsb")
                    nc.vector.tensor_copy(tailsb[:, 0:256], tailpv)
                    nc.vector.tensor_copy(tailsb[:, 256:512], tailst)
                    for hh in range(2):
                        h = 2 * p + hh
                        ops, stats = acc[hh]
                        nc.vector.tensor_add(
                            stats[:, 384:512], stats[:, 384:512],
                            tailsb[:, 256 + 128 * hh:384 + 128 * hh])
                        rec = rcp.tile([128, NQ], F32, name="rec")
                        nc.vector.reciprocal_approx_fast(out=rec, in_=stats)
                        nc.vector.tensor_add(
                            ops[:, 384:512], ops[:, 384:512],
                            tailsb[:, 128 * hh:128 * (hh + 1)])
                        nc.vector.tensor_mul(a_sb[h], ops, rec)

            # ---------------- P4: o_proj -----------------------------------
            with tc.tile_pool(name="wos", bufs=3) as wos, \
                 tc.tile_pool(name="wops", bufs=1, space="PSUM") as wops:
                for ec in range(4):
                    for qs_ in range(4):
                        opo = wops.tile([128, NQ], F32, name="opo", bufs=3)
                        for h in range(NH):
                            nc.tensor.matmul(
                                opo, a_sb[h][:, qs_ * 128:(qs_ + 1) * 128],
                                wo_t[ec][h], start=(h == 0),
                                stop=(h == NH - 1))
                        osb = wos.tile([128, NQ], F32, name="osb")
                        nc.vector.tensor_copy(osb, opo)
                        nc.sync.dma_start(
                            out_d[qs_ * 128:(qs_ + 1) * 128,
                                  ec * NQ:(ec + 1) * NQ], osb)
            wop_cm.__exit__(None, None, None)
    return nc


_MROPE_SECTION = [16, 24, 24]
_STREAM_IDX = np.concatenate(
    [np.full(n, i % 3, np.int64)
     for i, n in enumerate(_MROPE_SECTION * 2)])  # [128]


def _causal_mask_ok(mask):
    """True iff attention_mask is exactly the canonical causal mask."""
    m = np.asarray(mask, np.float32)
    if m.shape != (B, 1, S, S):
        return False
    tri = np.tril(np.ones((S, S), bool))
    canon = np.where(tri, np.float32(0.0), np.float32(NEG))
    return all(np.array_equal(m[b, 0], canon) for b in range(B))


def _host_prep_causal(hidden_states, cos, sin, Wq, bq, Wk, bk, Wv, bv, Wo):
    f = np.float32
    hs = np.asarray(hidden_states, f)
    cos = np.asarray(cos, f)
    sin = np.asarray(sin, f)
    ar = np.arange(D)

    wqP = _bf16(np.asarray(Wq, f).reshape(NH, D, HC_G, 128)
                .transpose(0, 3, 2, 1).reshape(HID, HID))
    shared = {
        "wqP": wqP,
        "wkvT": _bf16(np.concatenate([np.asarray(Wk, f).T,
                                      np.asarray(Wv, f).T], axis=1)),
        "woT": _bf16(np.asarray(Wo, f).T),
        "bqT": np.ascontiguousarray(np.asarray(bq, f).reshape(NH, D).T),
        "bkT": np.ascontiguousarray(np.asarray(bk, f).reshape(NKV, D).T),
        "bv": _bf16(np.asarray(bv, f).reshape(1, NKV * D)),
    }

    base = []
    for b in range(B):
        xT = hs[b].T                              # [HID, S]
        cosT = cos[_STREAM_IDX, b, :, ar]         # [128, S]
        sinT = sin[_STREAM_IDX, b, :, ar].copy()
        sinT[0:64, :] *= -1.0
        base.append((xT, cosT, sinT))

    i = np.arange(S)
    u, r = i // 4, i % 4
    t512 = np.arange(D)
    mm = np.arange(D)[:, None]                    # k within band tile

    in_maps = []
    for c in range(N_CORES):
        b, c4 = divmod(c, N_CORES // B)
        xT, cosT, sinT = base[b]
        gperm = 4 * u + (r + c4) % 4              # x' col -> global col
        m = dict(shared)
        m["xT"] = _bf16(xT[:, gperm])
        m["xq"] = _bf16(xT[:, c4::4])
        m["cosT"] = _round_fp32r(cosT[:, gperm])
        m["sinT"] = _round_fp32r(sinT[:, gperm])
        m["cosq"] = _round_fp32r(cosT[:, c4::4])
        m["sinq"] = _round_fp32r(sinT[:, c4::4])
        # maskq[k, band*128 + t]: key vs query within the diagonal band
        bands = []
        for band in range(4):
            mloc = 128 * band + mm                # in-band x' index
            kglob = 4 * (mloc // 4) + ((mloc % 4) + c4) % 4
            bands.append((kglob <= 4 * t512[None, :] + c4))
        m["maskq"] = _bf16(np.concatenate(bands, axis=1).astype(f))
        in_maps.append(m)
    return in_maps


HC_G = HID // 128


# ---------------------------------------------------------------------------
# dense fallback (original kernel, arbitrary additive mask via exp(mask))
# ---------------------------------------------------------------------------

def _build_nc(mm="f32r"):
    import contextlib
    import concourse.bass as bass  # noqa: F401
    import concourse.tile as tile
    from concourse import bacc, mybir

    F32 = mybir.dt.float32
    MMDT = mybir.dt.float32r if mm == "f32r" else F32

    nc = bacc.Bacc(target_bir_lowering=False, debug=False)

    def param(name, shape, dt=MMDT):
        return nc.declare_dram_parameter(name, list(shape), dt,
                                         isOutput=False)[:]

    xT = param("xT", [HID, S])
    wqT = param("wqT", [HID, HID])
    wkT = param("wkT", [HID, NKV * D])
    wvT = param("wvT", [HID, NKV * D])
    woT = param("woT", [HID, HID])
    bqT_d = param("bqT", [D, NH], F32)
    bkT_d = param("bkT", [D, NKV], F32)
    bv_d = param("bv", [1, NKV * D])
    cosT_d = param("cosT", [D, S])
    sinT_d = param("sinT", [D, S])
    cq_d = param("cosTq", [D, NQ])
    sq_d = param("sinTq", [D, NQ])
    maskT_d = param("maskT", [S, NQ])     # exp(mask).T, fp32r-rounded
    out_d = nc.declare_dram_parameter("out", [NQ, HID], F32, isOutput=True)[:]

    HC = HID // 128   # 16 contraction chunks
    KT = S // 128     # 16 key tiles
    KT2 = KT // 2     # 8 key tile-pairs
    TC = S // NQ      # 4 token chunks (for K/V proj)
    QS = NQ // 128    # 4 query sub-tiles

    Exp = mybir.ActivationFunctionType.Exp
    Ident = mybir.ActivationFunctionType.Identity

    lp = (nc.allow_low_precision(reason="fp32r matmul operands; psum stays f32")
          if mm == "f32r" else contextlib.nullcontext())
    with lp, tile.TileContext(nc) as tc:
        with tc.tile_pool(name="const", bufs=1) as cst, \
             tc.tile_pool(name="maskp", bufs=1) as maskp, \
             tc.tile_pool(name="kvp", bufs=1) as kvp:

            ones_row = cst.tile([1, 128], MMDT, name="ones_row")
            ones_sq = cst.tile([128, 128], MMDT, name="ones_sq")
            ones_f32 = cst.tile([128, 128], F32, name="ones_f32")
            nc.vector.memset(ones_f32, 1.0)
            nc.vector.tensor_copy(ones_row, ones_f32[0:1, :])
            nc.vector.tensor_copy(ones_sq, ones_f32)
            bqT = cst.tile([D, NH], F32, name="bqT")
            bkT = cst.tile([D, NKV], F32, name="bkT")
            bvr = cst.tile([1, NKV * D], MMDT, name="bvr")
            nc.sync.dma_start(bqT, bqT_d)
            nc.sync.dma_start(bkT, bkT_d)
            nc.sync.dma_start(bvr, bv_d)

            # exp(mask) tiles [128 k, 2 kt, 512 q], resident through attention
            mask_sb = [maskp.tile([128, 2, NQ], MMDT, name=f"mask{kt}")
                       for kt in range(KT2)]

            # token chunk 0 of xT = this core's query columns (host permutes
            # chunks); kept resident for the Q projection
            xq_sb = [kvp.tile([128, NQ], MMDT, name=f"xq{c}")
                     for c in range(HC)]
            # persistent K^T [d, t] per kv head; V [t, d] per token tile
            kT_sb = [kvp.tile([128, S], MMDT, name=f"kT{g}")
                     for g in range(NKV)]
            v_sb = [kvp.tile([128, NKV * D], MMDT, name=f"v{t}")
                    for t in range(KT)]

            # ---------------- P1a: K/V projection over all tokens ----------
            with tc.tile_pool(name="p1", bufs=1) as p1, \
                 tc.tile_pool(name="p1s", bufs=3) as p1s, \
                 tc.tile_pool(name="p1ps", bufs=1, space="PSUM") as p1ps:
                wk_sb = [p1.tile([128, NKV * D], MMDT, name=f"wk{c}")
                         for c in range(HC)]
                wv_sb = [p1.tile([128, NKV * D], MMDT, name=f"wv{c}")
                         for c in range(HC)]

                for tch in range(TC):
                    tsl = slice(tch * NQ, (tch + 1) * NQ)
                    kps = [p1ps.tile([128, NQ], F32, name=f"kps{g}", bufs=2)
                           for g in range(NKV)]
                    vps = [p1ps.tile([128, NKV * D], F32, name=f"vps{s_}",
                                     bufs=1) for s_ in range(4)]
                    for c in range(HC):
                        if tch == 0:
                            nc.sync.dma_start(wk_sb[c],
                                              wkT[c * 128:(c + 1) * 128, :])
                            nc.sync.dma_start(wv_sb[c],
                                              wvT[c * 128:(c + 1) * 128, :])
                            xt = xq_sb[c]
                        else:
                            xt = p1s.tile([128, NQ], MMDT, name="xt",
                                          bufs=8)
                        nc.sync.dma_start(xt, xT[c * 128:(c + 1) * 128, tsl])
                        for g in range(NKV):
                            nc.tensor.matmul(
                                kps[g], wk_sb[c][:, g * D:(g + 1) * D],
                                xt, start=(c == 0), stop=(c == HC - 1))
                        for s_ in range(4):
                            nc.tensor.matmul(
                                vps[s_], xt[:, s_ * 128:(s_ + 1) * 128],
                                wv_sb[c], start=(c == 0), stop=False)
                    # V bias via K=1 ones matmul, then evacuate
                    for s_ in range(4):
                        nc.tensor.matmul(vps[s_], ones_row, bvr,
                                         start=False, stop=True)
                        nc.vector.tensor_copy(v_sb[tch * 4 + s_], vps[s_])
                    # K bias + rope -> kT_sb
                    csb = p1s.tile([128, NQ], MMDT, name="csb")
                    ssb = p1s.tile([128, NQ], MMDT, name="ssb")
                    nc.sync.dma_start(csb, cosT_d[:, tsl])
                    nc.sync.dma_start(ssb, sinT_d[:, tsl])
                    for g in range(NKV):
                        kb = p1s.tile([128, NQ], MMDT, name="kb")
                        nc.scalar.activation(kb, kps[g], Ident,
                                             bias=bkT[:, g:g + 1])
                        ke = kT_sb[g][:, tsl]
                        shuf = p1s.tile([128, NQ], MMDT, name="shuf")
                        nc.sync.dma_start(shuf[0:64, :], kb[64:128, :])
                        nc.sync.dma_start(shuf[64:128, :], kb[0:64, :])
                        nc.vector.tensor_mul(ke, kb, csb)
                        nc.vector.tensor_mul(shuf, shuf, ssb)
                        nc.vector.tensor_add(ke, ke, shuf)

            # -------- P1b + P2: Q proj interleaved with attention ----------
            with tc.tile_pool(name="ap", bufs=1) as ap:
                a_sb = [ap.tile([128, NQ], MMDT, name=f"a{h}")
                        for h in range(NH)]
                with tc.tile_pool(name="p2", bufs=1) as p2, \
                     tc.tile_pool(name="p2s", bufs=2) as p2s, \
                     tc.tile_pool(name="p2w", bufs=10) as p2w, \
                     tc.tile_pool(name="qtp", bufs=2) as qtp, \
                     tc.tile_pool(name="att", bufs=5) as att, \
                     tc.tile_pool(name="atts", bufs=2) as atts:
                    cq = p2.tile([D, NQ], MMDT, name="cq")
                    sq = p2.tile([D, NQ], MMDT, name="sq")
                    nc.sync.dma_start(cq, cq_d)
                    nc.sync.dma_start(sq, sq_d)

                    for hg in range(4):
                        qT_sb = {}
                        with tc.tile_pool(name=f"qps{hg}", bufs=1,
                                          space="PSUM") as p2ps:
                            qps = [p2ps.tile([128, NQ], F32, name=f"qps{j}",
                                             bufs=1) for j in range(4)]
                            for c in range(HC):
                                wq = p2w.tile([128, NQ], MMDT, name="wq")
                                nc.sync.dma_start(
                                    wq, wqT[c * 128:(c + 1) * 128,
                                            hg * NQ:(hg + 1) * NQ])
                                for j in range(4):
                                    nc.tensor.matmul(
                                        qps[j], wq[:, j * 128:(j + 1) * 128],
                                        xq_sb[c], start=(c == 0),
                                        stop=(c == HC - 1))
                            for j in range(4):
                                h = hg * 4 + j
                                qT_sb[h] = qtp.tile([128, NQ], MMDT,
                                                    name=f"qT{j}")
                                qb = p2s.tile([128, NQ], MMDT, name="qb")
                                nc.scalar.activation(qb, qps[j], Ident,
                                                     bias=bqT[:, h:h + 1])
                                qe = qT_sb[h]
                                shufq = p2s.tile([128, NQ], MMDT,
                                                 name="shufq")
                                nc.sync.dma_start(shufq[0:64, :],
                                                  qb[64:128, :])
                                nc.sync.dma_start(shufq[64:128, :],
                                                  qb[0:64, :])
                                nc.vector.tensor_mul(qe, qb, cq)
                                nc.vector.tensor_mul(shufq, shufq, sq)
                                nc.vector.tensor_add(qe, qe, shufq)

                        if hg == 0:
                            for kt2 in range(KT2):
                                nc.sync.dma_start(
                                    mask_sb[kt2],
                                    maskT_d[256 * kt2:256 * (kt2 + 1),
                                            :].rearrange(
                                        "(a p) q -> p a q", a=2))
                        with tc.tile_pool(name=f"attps{hg}", bufs=1,
                                          space="PSUM") as attps:
                            for h in range(hg * 4, hg * 4 + 4):
                                g = h // (NH // NKV)
                                ops = attps.tile([128, NQ], F32, name="ops",
                                                 bufs=1)
                                stats = attps.tile([128, NQ], F32,
                                                   name="stats", bufs=1)
                                for kt2 in range(KT2):
                                    sps = attps.tile([128, 2, NQ], F32,
                                                     name="sps", bufs=3)
                                    ebuf = att.tile([128, 2, NQ], MMDT,
                                                    name="ebuf")
                                    for j2 in range(2):
                                        kt = 2 * kt2 + j2
                                        nc.tensor.matmul(
                                            sps[:, j2, :],
                                            kT_sb[g][:, kt * 128:
                                                     (kt + 1) * 128],
                                            qT_sb[h], start=True, stop=True)
                                    nc.scalar.activation(
                                        ebuf.rearrange("p a b -> p (a b)"),
                                        sps.rearrange("p a b -> p (a b)"),
                                        Exp, scale=float(SM_SCALE))
                                    nc.vector.tensor_mul(
                                        ebuf.rearrange("p a b -> p (a b)"),
                                        ebuf.rearrange("p a b -> p (a b)"),
                                        mask_sb[kt2].rearrange(
                                            "p a b -> p (a b)"))
                                    for j2 in range(2):
                                        kt = 2 * kt2 + j2
                                        nc.tensor.matmul(
                                            stats, ones_sq, ebuf[:, j2, :],
                                            start=(kt == 0),
                                            stop=(kt == KT - 1))
                                        nc.tensor.matmul(
                                            ops,
                                            v_sb[kt][:, g * D:(g + 1) * D],
                                            ebuf[:, j2, :],
                                            start=(kt == 0),
                                            stop=(kt == KT - 1))
                                recip = atts.tile([128, NQ], F32,
                                                  name="recip")
                                nc.vector.reciprocal_approx_fast(
                                    out=recip, in_=stats)
                                nc.vector.tensor_mul(a_sb[h], ops, recip)

                # ------------- P3: o_proj ------------------------------
                with tc.tile_pool(name="wop", bufs=1) as wop, \
                     tc.tile_pool(name="wos", bufs=3) as wos, \
                     tc.tile_pool(name="wops", bufs=1, space="PSUM") as wops:
                    for ec in range(4):
                        wo_t = [wop.tile([128, NQ], MMDT, name=f"wo{h}",
                                         bufs=2) for h in range(NH)]
                        for h in range(NH):
                            nc.sync.dma_start(
                                wo_t[h], woT[h * 128:(h + 1) * 128,
                                             ec * NQ:(ec + 1) * NQ])
                        for qs_ in range(QS):
                            opo = wops.tile([128, NQ], F32, name="opo",
                                            bufs=3)
                            for h in range(NH):
                                nc.tensor.matmul(
                                    opo,
                                    a_sb[h][:, qs_ * 128:(qs_ + 1) * 128],
                                    wo_t[h], start=(h == 0),
                                    stop=(h == NH - 1))
                            osb = wos.tile([128, NQ], F32, name="osb")
                            nc.vector.tensor_copy(osb, opo)
                            nc.sync.dma_start(
                                out_d[qs_ * 128:(qs_ + 1) * 128,
                                      ec * NQ:(ec + 1) * NQ], osb)
    return nc


def get_nc(key):
    if key not in _BUILD_CACHE:
        if key == "causal":
            nc = _build_causal()
        else:
            nc = _build_nc(key)
        nc.finalize()
        _BUILD_CACHE[key] = nc
    return _BUILD_CACHE[key]


def _host_prep(hidden_states, cos, sin, attention_mask, Wq, bq, Wk, bk, Wv,
               bv, Wo, mm="f32r"):
    f = np.float32
    if mm == "f32r":
        rnd = _round_fp32r
    else:
        def rnd(a):
            return np.ascontiguousarray(a, f)
    hs = np.asarray(hidden_states, f)
    cos = np.asarray(cos, f)
    sin = np.asarray(sin, f)
    mask = np.asarray(attention_mask, f)
    ar = np.arange(D)

    shared = {
        "wqT": rnd(np.asarray(Wq, f).T),
        "wkT": rnd(np.asarray(Wk, f).T),
        "wvT": rnd(np.asarray(Wv, f).T),
        "woT": rnd(np.asarray(Wo, f).T),
        "bqT": np.ascontiguousarray(np.asarray(bq, f).reshape(NH, D).T),
        "bkT": np.ascontiguousarray(np.asarray(bk, f).reshape(NKV, D).T),
        "bv": rnd(np.asarray(bv, f).reshape(1, NKV * D)),
    }

    per_batch = []
    for b in range(B):
        xT = rnd(hs[b].T)
        cosT = rnd(cos[_STREAM_IDX, b, :, ar])  # [128, S]
        sinT = rnd(sin[_STREAM_IDX, b, :, ar])
        sinT[0:64, :] *= -1.0   # rotate_half sign folded into sin
        maskT = rnd(np.exp(mask[b, 0].T.astype(np.float64)
                           ).astype(np.float32))
        per_batch.append((xT, cosT, sinT, maskT))

    in_maps = []
    for c in range(N_CORES):
        b, qc = divmod(c, N_CORES // B)
        xT, cosT, sinT, maskT = per_batch[b]
        qsl = slice(qc * NQ, (qc + 1) * NQ)
        order = [qc] + [o for o in range(N_CORES // B) if o != qc]
        tperm = np.concatenate([np.arange(o * NQ, (o + 1) * NQ)
                                for o in order])
        m = dict(shared)
        m["xT"] = np.ascontiguousarray(xT[:, tperm])
        m["cosT"] = np.ascontiguousarray(cosT[:, tperm])
        m["sinT"] = np.ascontiguousarray(sinT[:, tperm])
        m["maskT"] = np.ascontiguousarray(maskT[tperm][:, qsl])
        m["cosTq"] = np.ascontiguousarray(cosT[:, qsl])
        m["sinTq"] = np.ascontiguousarray(sinT[:, qsl])
        in_maps.append(m)
    return in_maps


def kernel(hidden_states, cos, sin, attention_mask, Wq, bq, Wk, bk, Wv, bv,
           Wo, _trace=False, _mm="causal"):
    from concourse.bass_utils import run_bass_kernel_spmd

    if _mm == "causal" and not _causal_mask_ok(attention_mask):
        _mm = "f32r"

    if _mm == "causal":
        in_maps = _host_prep_causal(hidden_states, cos, sin, Wq, bq, Wk, bk,
                                    Wv, bv, Wo)
        nc = get_nc("causal")
        res = run_bass_kernel_spmd(nc, in_maps, list(range(N_CORES)),
                                   trace=_trace)
        out = np.empty((B, S, HID), np.float32)
        for c in range(N_CORES):
            b, c4 = divmod(c, N_CORES // B)
            out[b, c4::4, :] = res.results[c]["out"]
        kernel._last_results = res
        return out

    in_maps = _host_prep(hidden_states, cos, sin, attention_mask, Wq, bq, Wk,
                         bk, Wv, bv, Wo, mm=_mm)
    nc = get_nc(_mm)
    res = run_bass_kernel_spmd(nc, in_maps, list(range(N_CORES)),
                               trace=_trace)
    out = np.empty((B, S, HID), np.float32)
    for c in range(N_CORES):
        b, qc = divmod(c, N_CORES // B)
        out[b, qc * NQ:(qc + 1) * NQ, :] = res.results[c]["out"]
    kernel._last_results = res
    return out


# revision 28
# speedup vs baseline: 1.5742x; 1.0427x over previous
"""Qwen2.5-VL attention (mrope + GQA + causal mask + o_proj) on 8 Trainium2
NeuronCores.

Fast path (causal mask detected): batch x interleaved-query sharding.
Core c handles batch b = c//4 and the stride-4 query comb {c4, c4+4, ...}
(c4 = c%4).  Host permutes tokens within every aligned group of 4 so the
core's queries sit at on-device columns 0::4; key tiles of 128 then cover
the same global token sets on every core, giving an SPMD-uniform causal
structure: query sub-tile j (128 rows, global span [512j, 512j+512)) only
attends key tiles kt < 4(j+1).

Attention loop per head: 12 "main" key tiles with shrinking query windows
(512/384/256 wide, always >=256 so fp32r/bf16 matmuls stay 1 cycle/row)
plus a 4-tile diagonal "tail" computed jointly for a head PAIR (the two
heads' last 128-query windows concatenated to free dim 256).  Only the
single diagonal key tile per iteration needs the 0/1 mask multiply; fully
masked tiles are skipped, fully visible tiles skip the multiply.

Softmax denominators via ones[128,128] matmuls accumulated region-wise in
PSUM alongside PV (start/stop flags per region, skip_group_check).

dtypes: projections stream bf16 (x, Wq, Wk, Wv, Wo); rope + scores run in
fp32r (cos/sin, qT, kT tiles); exp output / V / attn weights bf16; all
PSUM accumulation f32.  Measured rel err ~6e-3 vs the f32 reference.

Fallback (non-causal mask): the original dense kernel (exp(mask)
multiplicative masking over all tiles, fp32r everywhere).
"""

import sys

for _p in ("/opt/trn_rl_repo", "/root/.axon_site/_ro/trn_rl_repo"):
    if _p not in sys.path:
        sys.path.insert(0, _p)

import numpy as np

B = 2
S = 2048
HID = 2048
NH = 16
NKV = 2
D = 128
NQ = 512          # query rows per core
N_CORES = 8
SM_SCALE = 1.0 / np.sqrt(np.float32(D))
NEG = -3.4028235e38

_BUILD_CACHE = {}


def _round_fp32r(a):
    """Round-to-nearest-even to 12 explicit mantissa bits (fp32r)."""
    u = np.ascontiguousarray(a, np.float32).view(np.uint32)
    low = u & np.uint32(0xFFF)
    up = (u & np.uint32(0xFFFFF000)) + np.uint32(0x1000)
    half = low == np.uint32(0x800)
    rnd = np.where(low > 0x800, up,
                   np.where(half & ((u & np.uint32(0x1000)) != 0), up,
                            u & np.uint32(0xFFFFF000)))
    expmask = (u & np.uint32(0x7F800000)) == np.uint32(0x7F800000)
    rnd = np.where(expmask, u, rnd)
    return rnd.view(np.float32)


def _bf16(a):
    import ml_dtypes
    return np.asarray(a, np.float32).astype(ml_dtypes.bfloat16)


# ---------------------------------------------------------------------------
# causal fast path
# ---------------------------------------------------------------------------

def _build_causal():
    import concourse.bass as bass  # noqa: F401 (env init)
    import concourse.tile as tile
    from concourse import bacc, mybir

    F32 = mybir.dt.float32
    F32R = mybir.dt.float32r
    BF16 = mybir.dt.bfloat16

    nc = bacc.Bacc(target_bir_lowering=False, debug=False)

    def param(name, shape, dt):
        return nc.declare_dram_parameter(name, list(shape), dt,
                                         isOutput=False)[:]

    xT_d = param("xT", [HID, S], BF16)          # permuted token order
    xq_d = param("xq", [HID, NQ], BF16)         # this core's query columns
    wqP_d = param("wqP", [HID, HID], BF16)      # [h*128+k, c*128+m]
    wkvT_d = param("wkvT", [HID, 2 * NKV * D], BF16)   # [Wk.T | Wv.T]
    woT_d = param("woT", [HID, HID], BF16)
    bqT_d = param("bqT", [D, NH], F32)
    bkT_d = param("bkT", [D, NKV], F32)
    bv_d = param("bv", [1, NKV * D], BF16)
    cosT_d = param("cosT", [D, S], F32R)        # permuted, mrope-merged
    sinT_d = param("sinT", [D, S], F32R)        # top half sign-flipped
    cosq_d = param("cosq", [D, NQ], F32R)       # query columns
    sinq_d = param("sinq", [D, NQ], F32R)
    maskq_d = param("maskq", [D, 4 * D], BF16)  # diag band 0/1, j-invariant
    out_d = nc.declare_dram_parameter("out", [NQ, HID], F32, isOutput=True)[:]

    HC = HID // 128   # 16 contraction chunks
    KT = S // 128     # 16 key tiles
    TC = S // NQ      # 4 token chunks
    NP = NH // 2      # 8 head pairs

    Exp = mybir.ActivationFunctionType.Exp
    Ident = mybir.ActivationFunctionType.Identity

    with nc.allow_low_precision(reason="bf16/fp32r matmuls; psum stays f32"), \
         tile.TileContext(nc) as tc:
        with tc.tile_pool(name="const", bufs=1) as cst, \
             tc.tile_pool(name="kvp", bufs=1) as kvp, \
             tc.tile_pool(name="qtp", bufs=1) as qtp:

            ones_f32 = cst.tile([128, 128], F32, name="ones_f32")
            ones_sq = cst.tile([128, 128], BF16, name="ones_sq")
            ones_row = cst.tile([1, 128], BF16, name="ones_row")
            nc.vector.memset(ones_f32, 1.0)
            nc.vector.tensor_copy(ones_sq, ones_f32)
            nc.vector.tensor_copy(ones_row, ones_f32[0:1, :])
            bqT = cst.tile([D, NH], F32, name="bqT")
            bkT = cst.tile([D, NKV], F32, name="bkT")
            bvr = cst.tile([1, NKV * D], BF16, name="bvr")
            maskq = cst.tile([D, 4 * D], BF16, name="maskq")
            cosT = cst.tile([D, S], F32R, name="cosT")
            sinT = cst.tile([D, S], F32R, name="sinT")
            cosq = cst.tile([D, NQ], F32R, name="cosq")
            sinq = cst.tile([D, NQ], F32R, name="sinq")
            nc.sync.dma_start(bqT, bqT_d)
            nc.sync.dma_start(bkT, bkT_d)
            nc.sync.dma_start(bvr, bv_d)
            nc.sync.dma_start(maskq, maskq_d)
            nc.sync.dma_start(cosT, cosT_d)
            nc.sync.dma_start(sinT, sinT_d)
            nc.sync.dma_start(cosq, cosq_d)
            nc.sync.dma_start(sinq, sinq_d)

            # persistent K^T [d, t] per kv head; V [t, 2*d] per token tile
            kT_sb = [kvp.tile([128, S], F32R, name=f"kT{g}")
                     for g in range(NKV)]
            v_sb = [kvp.tile([128, NKV * D], BF16, name=f"v{t}")
                    for t in range(KT)]
            # rope'd Q per head pair: [d, 2 heads, 512 q]
            qT2 = [qtp.tile([128, 2, NQ], F32R, name=f"qT2_{p}")
                   for p in range(NP)]
            # both heads' last 256-query window (kt 8..11 merged scores);
            # the diagonal tail (last 128) is a strided slice of this
            qtl2 = [qtp.tile([128, 512], F32R, name=f"qtl2_{p}")
                    for p in range(NP)]

            # strided views: the core's query columns are x' cols 0::4
            def qcols(t):
                return t.rearrange("p (u r) -> p u r", r=4)[:, :, 0]

            with tc.tile_pool(name="xp", bufs=1) as xp:
                x_sb = [xp.tile([128, S], BF16, name=f"x{c}")
                        for c in range(HC)]
                xq_sb = [xp.tile([128, NQ], BF16, name=f"xq{c}")
                         for c in range(HC)]

                # ------------- P1: K/V projection over all tokens ----------
                with tc.tile_pool(name="p1w", bufs=1) as p1w, \
                     tc.tile_pool(name="p1s", bufs=2) as p1s, \
                     tc.tile_pool(name="p1ps", bufs=1, space="PSUM") as p1ps:
                    wkv_sb = [p1w.tile([128, 2 * NKV * D], BF16,
                                       name=f"wkv{c}") for c in range(HC)]
                    for c in range(HC):
                        nc.gpsimd.dma_start(
                            x_sb[c], xT_d[c * 128:(c + 1) * 128, :])
                        nc.gpsimd.dma_start(
                            wkv_sb[c], wkvT_d[c * 128:(c + 1) * 128, :])
                    for c in range(HC):
                        nc.gpsimd.dma_start(
                            xq_sb[c], xq_d[c * 128:(c + 1) * 128, :])
                    for tch in range(TC):
                        tsl = slice(tch * NQ, (tch + 1) * NQ)
                        kps = [p1ps.tile([128, NQ], F32, name=f"kps{g}",
                                         bufs=2) for g in range(NKV)]
                        vps = [p1ps.tile([128, NKV * D], F32, name=f"vps{s_}",
                                         bufs=1) for s_ in range(4)]
                        for c in range(HC):
                            for g in range(NKV):
                                nc.tensor.matmul(
                                    kps[g], wkv_sb[c][:, g * D:(g + 1) * D],
                                    x_sb[c][:, tsl],
                                    start=(c == 0), stop=(c == HC - 1))
                            for s_ in range(4):
                                nc.tensor.matmul(
                                    vps[s_],
                                    x_sb[c][:, tch * NQ + s_ * 128:
                                            tch * NQ + (s_ + 1) * 128],
                                    wkv_sb[c][:, NKV * D:2 * NKV * D],
                                    start=(c == 0), stop=False)
                        for s_ in range(4):
                            nc.tensor.matmul(vps[s_], ones_row, bvr,
                                             start=False, stop=True)
                            nc.vector.tensor_copy(v_sb[tch * 4 + s_], vps[s_])
                        for g in range(NKV):
                            kb = p1s.tile([128, NQ], F32R, name="kb")
                            nc.scalar.activation(kb, kps[g], Ident,
                                                 bias=bkT[:, g:g + 1])
                            shuf = p1s.tile([128, NQ], F32R, name="shuf")
                            nc.sync.dma_start(shuf[0:64, :], kb[64:128, :])
                            nc.sync.dma_start(shuf[64:128, :], kb[0:64, :])
                            ke = kT_sb[g][:, tsl]
                            nc.vector.tensor_mul(ke, kb, cosT[:, tsl])
                            nc.vector.tensor_mul(shuf, shuf, sinT[:, tsl])
                            nc.vector.tensor_add(ke, ke, shuf)

                # ------------- P2: Q projection + rope ---------------------
                with tc.tile_pool(name="p2w", bufs=3) as p2w, \
                     tc.tile_pool(name="p2s", bufs=2) as p2s, \
                     tc.tile_pool(name="p2ps", bufs=1, space="PSUM") as p2ps:
                    for h in range(NH):
                        wqh = p2w.tile([128, HID], BF16, name="wqh")
                        nc.gpsimd.dma_start(wqh,
                                            wqP_d[h * 128:(h + 1) * 128, :])
                        qps = p2ps.tile([128, NQ], F32, name="qps", bufs=4)
                        for c in range(HC):
                            nc.tensor.matmul(
                                qps, wqh[:, c * 128:(c + 1) * 128],
                                xq_sb[c],
                                start=(c == 0), stop=(c == HC - 1))
                        qb = p2s.tile([128, NQ], F32R, name="qb")
                        nc.scalar.activation(qb, qps, Ident,
                                             bias=bqT[:, h:h + 1])
                        shufq = p2s.tile([128, NQ], F32R, name="shufq")
                        nc.sync.dma_start(shufq[0:64, :], qb[64:128, :])
                        nc.sync.dma_start(shufq[64:128, :], qb[0:64, :])
                        qe = qT2[h // 2][:, h % 2, :]
                        nc.vector.tensor_mul(qe, qb, cosq)
                        nc.vector.tensor_mul(shufq, shufq, sinq)
                        nc.vector.tensor_add(qe, qe, shufq)
                        nc.vector.tensor_copy(
                            qtl2[h // 2][:, (h % 2) * 256:(h % 2 + 1) * 256],
                            qe[:, 256:512])

            # ---------------- P3: causal attention -------------------------
            # (o_proj weights prefetch during attention via the idle Pool
            #  DMA queue; the wop pool outlives P3 into P4)
            ap_cm = tc.tile_pool(name="ap", bufs=1)
            ap = ap_cm.__enter__()
            a_sb = [ap.tile([128, NQ], BF16, name=f"a{h}")
                    for h in range(NH)]
            wop_cm = tc.tile_pool(name="wop", bufs=1)
            wop = wop_cm.__enter__()
            wo_t = [[wop.tile([128, NQ], BF16, name=f"wo{ec}_{h}")
                     for h in range(NH)] for ec in range(4)]
            for ec in range(4):
                for h in range(NH):
                    nc.sync.dma_start(
                        wo_t[ec][h], woT_d[h * 128:(h + 1) * 128,
                                           ec * NQ:(ec + 1) * NQ])
            with tc.tile_pool(name="ebp", bufs=3) as ebp, \
                 tc.tile_pool(name="rcp", bufs=2) as rcp, \
                 tc.tile_pool(name="spsp", bufs=1, space="PSUM") as spsp, \
                 tc.tile_pool(name="opsp", bufs=1, space="PSUM") as opsp, \
                 tc.tile_pool(name="stpsp", bufs=1, space="PSUM") as stpsp, \
                 tc.tile_pool(name="tlpsp", bufs=1, space="PSUM") as tlpsp:
                for p in range(NP):
                    g = p // (NP // NKV)
                    acc = [(opsp.tile([128, NQ], F32, name="ops", bufs=2),
                            stpsp.tile([128, NQ], F32, name="stats", bufs=2))
                           for _ in range(2)]
                    # kt 0..7: per-head, query window shrinks 512 -> 384
                    for kt in range(8):
                        off = 128 * (kt // 4)
                        w = slice(off, NQ)
                        bnd = (kt % 4) * 128
                        kslice = kT_sb[g][:, kt * 128:(kt + 1) * 128]
                        for hh in range(2):
                            ops, stats = acc[hh]
                            sps = spsp.tile([128, NQ], F32, name="sps",
                                            bufs=2)
                            nc.tensor.matmul(
                                sps[:, w], kslice, qT2[p][:, hh, w],
                                start=True, stop=True)
                            eb = ebp.tile([128, NQ], BF16, name="eb")
                            nc.scalar.activation(eb[:, w], sps[:, w], Exp,
                                                 scale=float(SM_SCALE))
                            nc.vector.tensor_mul(
                                eb[:, off:off + 128], eb[:, off:off + 128],
                                maskq[:, bnd:bnd + 128])
                            nc.tensor.matmul(
                                stats[:, w], ones_sq, eb[:, w],
                                start=(kt == 0), stop=False,
                                skip_group_check=True)
                            nc.tensor.matmul(
                                ops[:, w], v_sb[kt][:, g * D:(g + 1) * D],
                                eb[:, w], start=(kt == 0), stop=False,
                                skip_group_check=True)
                    # kt 8..11: both heads' last 256 queries in one matmul
                    for kt in range(8, 12):
                        bnd = (kt % 4) * 128
                        sps = spsp.tile([128, NQ], F32, name="sps", bufs=2)
                        nc.tensor.matmul(
                            sps, kT_sb[g][:, kt * 128:(kt + 1) * 128],
                            qtl2[p], start=True, stop=True)
                        eb = ebp.tile([128, NQ], BF16, name="eb")
                        nc.scalar.activation(eb, sps, Exp,
                                             scale=float(SM_SCALE))
                        nc.vector.tensor_mul(eb[:, 0:128], eb[:, 0:128],
                                             maskq[:, bnd:bnd + 128])
                        nc.vector.tensor_mul(eb[:, 256:384], eb[:, 256:384],
                                             maskq[:, bnd:bnd + 128])
                        for hh in range(2):
                            ops, stats = acc[hh]
                            sl = slice(256 * hh, 256 * hh + 256)
                            nc.tensor.matmul(
                                stats[:, 256:512], ones_sq, eb[:, sl],
                                start=False, stop=(kt == 11),
                                skip_group_check=True)
                            nc.tensor.matmul(
                                ops[:, 256:512],
                                v_sb[kt][:, g * D:(g + 1) * D],
                                eb[:, sl], start=False, stop=(kt == 11),
                                skip_group_check=True)
                    # diagonal tail: both heads' last 128-query window
                    tailpv = tlpsp.tile([128, 256], F32, name="tailpv",
                                        bufs=1)
                    tailst = tlpsp.tile([128, 256], F32, name="tailst",
                                        bufs=1)
                    qtail = qtl2[p].rearrange("p (s a b) -> p s a b",
                                              s=2, a=2)[:, :, 1, :]
                    for kt in range(12, 16):
                        sps = spsp.tile([128, NQ], F32, name="sps", bufs=2)
                        nc.tensor.matmul(
                            sps[:, 0:256],
                            kT_sb[g][:, kt * 128:(kt + 1) * 128],
                            qtail, start=True, stop=True)
                        eb = ebp.tile([128, NQ], BF16, name="eb")
                        nc.scalar.activation(eb[:, 0:256], sps[:, 0:256],
                                             Exp, scale=float(SM_SCALE))
                        bnd = (kt % 4) * 128
                        nc.vector.tensor_mul(eb[:, 0:128], eb[:, 0:128],
                                             maskq[:, bnd:bnd + 128])
                        nc.vector.tensor_mul(eb[:, 128:256], eb[:, 128:256],
                                             maskq[:, bnd:bnd + 128])
                        nc.tensor.matmul(tailst, ones_sq,
                                         eb[:, 0:256], start=(kt == 12),
                                         stop=(kt == 15),
                                         skip_group_check=True)
                        nc.tensor.matmul(tailpv,
                                         v_sb[kt][:, g * D:(g + 1) * D],
                                         eb[:, 0:256], start=(kt == 12),
                                         stop=(kt == 15),
                                         skip_group_check=True)
                    tailsb = rcp.tile([128, NQ], F32, name="tailsb")
                    nc.vector.tensor_copy(tailsb[:, 0:256], tailpv)
                    nc.vector.tensor_copy(tailsb[:, 256:512], tailst)
                    for hh in range(2):
                        h = 2 * p + hh
                        ops, stats = acc[hh]
                        nc.vector.tensor_add(
                            stats[:, 384:512], stats[:, 384:512],
                            tailsb[:, 256 + 128 * hh:384 + 128 * hh])
                        rec = rcp.tile([128, NQ], F32, name="rec")
                        nc.vector.reciprocal_approx_fast(out=rec, in_=stats)
                        nc.vector.tensor_add(
                            ops[:, 384:512], ops[:, 384:512],
                            tailsb[:, 128 * hh:128 * (hh + 1)])
                        nc.vector.tensor_mul(a_sb[h], ops, rec)

            # ---------------- P4: o_proj -----------------------------------
            with tc.tile_pool(name="wos", bufs=3) as wos, \
                 tc.tile_pool(name="wops", bufs=1, space="PSUM") as wops:
                for ec in range(4):
                    for qs_ in range(4):
                        opo = wops.tile([128, NQ], F32, name="opo", bufs=3)
                        for h in range(NH):
                            nc.tensor.matmul(
                                opo, a_sb[h][:, qs_ * 128:(qs_ + 1) * 128],
                                wo_t[ec][h], start=(h == 0),
                                stop=(h == NH - 1))
                        osb = wos.tile([128, NQ], F32, name="osb")
                        nc.vector.tensor_copy(osb, opo)
                        nc.sync.dma_start(
                            out_d[qs_ * 128:(qs_ + 1) * 128,
                                  ec * NQ:(ec + 1) * NQ], osb)
            wop_cm.__exit__(None, None, None)
            ap_cm.__exit__(None, None, None)
    return nc


_MROPE_SECTION = [16, 24, 24]
_STREAM_IDX = np.concatenate(
    [np.full(n, i % 3, np.int64)
     for i, n in enumerate(_MROPE_SECTION * 2)])  # [128]


def _causal_mask_ok(mask):
    """True iff attention_mask is exactly the canonical causal mask."""
    m = np.asarray(mask, np.float32)
    if m.shape != (B, 1, S, S):
        return False
    tri = np.tril(np.ones((S, S), bool))
    canon = np.where(tri, np.float32(0.0), np.float32(NEG))
    return all(np.array_equal(m[b, 0], canon) for b in range(B))


def _host_prep_causal(hidden_states, cos, sin, Wq, bq, Wk, bk, Wv, bv, Wo):
    f = np.float32
    hs = np.asarray(hidden_states, f)
    cos = np.asarray(cos, f)
    sin = np.asarray(sin, f)
    ar = np.arange(D)

    wqP = _bf16(np.asarray(Wq, f).reshape(NH, D, HC_G, 128)
                .transpose(0, 3, 2, 1).reshape(HID, HID))
    shared = {
        "wqP": wqP,
        "wkvT": _bf16(np.concatenate([np.asarray(Wk, f).T,
                                      np.asarray(Wv, f).T], axis=1)),
        "woT": _bf16(np.asarray(Wo, f).T),
        "bqT": np.ascontiguousarray(np.asarray(bq, f).reshape(NH, D).T),
        "bkT": np.ascontiguousarray(np.asarray(bk, f).reshape(NKV, D).T),
        "bv": _bf16(np.asarray(bv, f).reshape(1, NKV * D)),
    }

    base = []
    for b in range(B):
        xT = hs[b].T                              # [HID, S]
        cosT = cos[_STREAM_IDX, b, :, ar]         # [128, S]
        sinT = sin[_STREAM_IDX, b, :, ar].copy()
        sinT[0:64, :] *= -1.0
        base.append((xT, cosT, sinT))

    i = np.arange(S)
    u, r = i // 4, i % 4
    t512 = np.arange(D)
    mm = np.arange(D)[:, None]                    # k within band tile

    in_maps = []
    for c in range(N_CORES):
        b, c4 = divmod(c, N_CORES // B)
        xT, cosT, sinT = base[b]
        gperm = 4 * u + (r + c4) % 4              # x' col -> global col
        m = dict(shared)
        m["xT"] = _bf16(xT[:, gperm])
        m["xq"] = _bf16(xT[:, c4::4])
        m["cosT"] = _round_fp32r(cosT[:, gperm])
        m["sinT"] = _round_fp32r(sinT[:, gperm])
        m["cosq"] = _round_fp32r(cosT[:, c4::4])
        m["sinq"] = _round_fp32r(sinT[:, c4::4])
        # maskq[k, band*128 + t]: key vs query within the diagonal band
        bands = []
        for band in range(4):
            mloc = 128 * band + mm                # in-band x' index
            kglob = 4 * (mloc // 4) + ((mloc % 4) + c4) % 4
            bands.append((kglob <= 4 * t512[None, :] + c4))
        m["maskq"] = _bf16(np.concatenate(bands, axis=1).astype(f))
        in_maps.append(m)
    return in_maps


HC_G = HID // 128


# ---------------------------------------------------------------------------
# dense fallback (original kernel, arbitrary additive mask via exp(mask))
# ---------------------------------------------------------------------------

def _build_nc(mm="f32r"):
    import contextlib
    import concourse.bass as bass  # noqa: F401
    import concourse.tile as tile
    from concourse import bacc, mybir

    F32 = mybir.dt.float32
    MMDT = mybir.dt.float32r if mm == "f32r" else F32

    nc = bacc.Bacc(target_bir_lowering=False, debug=False)

    def param(name, shape, dt=MMDT):
        return nc.declare_dram_parameter(name, list(shape), dt,
                                         isOutput=False)[:]

    xT = param("xT", [HID, S])
    wqT = param("wqT", [HID, HID])
    wkT = param("wkT", [HID, NKV * D])
    wvT = param("wvT", [HID, NKV * D])
    woT = param("woT", [HID, HID])
    bqT_d = param("bqT", [D, NH], F32)
    bkT_d = param("bkT", [D, NKV], F32)
    bv_d = param("bv", [1, NKV * D])
    cosT_d = param("cosT", [D, S])
    sinT_d = param("sinT", [D, S])
    cq_d = param("cosTq", [D, NQ])
    sq_d = param("sinTq", [D, NQ])
    maskT_d = param("maskT", [S, NQ])     # exp(mask).T, fp32r-rounded
    out_d = nc.declare_dram_parameter("out", [NQ, HID], F32, isOutput=True)[:]

    HC = HID // 128   # 16 contraction chunks
    KT = S // 128     # 16 key tiles
    KT2 = KT // 2     # 8 key tile-pairs
    TC = S // NQ      # 4 token chunks (for K/V proj)
    QS = NQ // 128    # 4 query sub-tiles

    Exp = mybir.ActivationFunctionType.Exp
    Ident = mybir.ActivationFunctionType.Identity

    lp = (nc.allow_low_precision(reason="fp32r matmul operands; psum stays f32")
          if mm == "f32r" else contextlib.nullcontext())
    with lp, tile.TileContext(nc) as tc:
        with tc.tile_pool(name="const", bufs=1) as cst, \
             tc.tile_pool(name="maskp", bufs=1) as maskp, \
             tc.tile_pool(name="kvp", bufs=1) as kvp:

            ones_row = cst.tile([1, 128], MMDT, name="ones_row")
            ones_sq = cst.tile([128, 128], MMDT, name="ones_sq")
            ones_f32 = cst.tile([128, 128], F32, name="ones_f32")
            nc.vector.memset(ones_f32, 1.0)
            nc.vector.tensor_copy(ones_row, ones_f32[0:1, :])
            nc.vector.tensor_copy(ones_sq, ones_f32)
            bqT = cst.tile([D, NH], F32, name="bqT")
            bkT = cst.tile([D, NKV], F32, name="bkT")
            bvr = cst.tile([1, NKV * D], MMDT, name="bvr")
            nc.sync.dma_start(bqT, bqT_d)
            nc.sync.dma_start(bkT, bkT_d)
            nc.sync.dma_start(bvr, bv_d)

            # exp(mask) tiles [128 k, 2 kt, 512 q], resident through attention
            mask_sb = [maskp.tile([128, 2, NQ], MMDT, name=f"mask{kt}")
                       for kt in range(KT2)]

            # token chunk 0 of xT = this core's query columns (host permutes
            # chunks); kept resident for the Q projection
            xq_sb = [kvp.tile([128, NQ], MMDT, name=f"xq{c}")
                     for c in range(HC)]
            # persistent K^T [d, t] per kv head; V [t, d] per token tile
            kT_sb = [kvp.tile([128, S], MMDT, name=f"kT{g}")
                     for g in range(NKV)]
            v_sb = [kvp.tile([128, NKV * D], MMDT, name=f"v{t}")
                    for t in range(KT)]

            # ---------------- P1a: K/V projection over all tokens ----------
            with tc.tile_pool(name="p1", bufs=1) as p1, \
                 tc.tile_pool(name="p1s", bufs=3) as p1s, \
                 tc.tile_pool(name="p1ps", bufs=1, space="PSUM") as p1ps:
                wk_sb = [p1.tile([128, NKV * D], MMDT, name=f"wk{c}")
                         for c in range(HC)]
                wv_sb = [p1.tile([128, NKV * D], MMDT, name=f"wv{c}")
                         for c in range(HC)]

                for tch in range(TC):
                    tsl = slice(tch * NQ, (tch + 1) * NQ)
                    kps = [p1ps.tile([128, NQ], F32, name=f"kps{g}", bufs=2)
                           for g in range(NKV)]
                    vps = [p1ps.tile([128, NKV * D], F32, name=f"vps{s_}",
                                     bufs=1) for s_ in range(4)]
                    for c in range(HC):
                        if tch == 0:
                            nc.sync.dma_start(wk_sb[c],
                                              wkT[c * 128:(c + 1) * 128, :])
                            nc.sync.dma_start(wv_sb[c],
                                              wvT[c * 128:(c + 1) * 128, :])
                            xt = xq_sb[c]
                        else:
                            xt = p1s.tile([128, NQ], MMDT, name="xt",
                                          bufs=8)
                        nc.sync.dma_start(xt, xT[c * 128:(c + 1) * 128, tsl])
                        for g in range(NKV):
                            nc.tensor.matmul(
                                kps[g], wk_sb[c][:, g * D:(g + 1) * D],
                                xt, start=(c == 0), stop=(c == HC - 1))
                        for s_ in range(4):
                            nc.tensor.matmul(
                                vps[s_], xt[:, s_ * 128:(s_ + 1) * 128],
                                wv_sb[c], start=(c == 0), stop=False)
                    # V bias via K=1 ones matmul, then evacuate
                    for s_ in range(4):
                        nc.tensor.matmul(vps[s_], ones_row, bvr,
                                         start=False, stop=True)
                        nc.vector.tensor_copy(v_sb[tch * 4 + s_], vps[s_])
                    # K bias + rope -> kT_sb
                    csb = p1s.tile([128, NQ], MMDT, name="csb")
                    ssb = p1s.tile([128, NQ], MMDT, name="ssb")
                    nc.sync.dma_start(csb, cosT_d[:, tsl])
                    nc.sync.dma_start(ssb, sinT_d[:, tsl])
                    for g in range(NKV):
                        kb = p1s.tile([128, NQ], MMDT, name="kb")
                        nc.scalar.activation(kb, kps[g], Ident,
                                             bias=bkT[:, g:g + 1])
                        ke = kT_sb[g][:, tsl]
                        shuf = p1s.tile([128, NQ], MMDT, name="shuf")
                        nc.sync.dma_start(shuf[0:64, :], kb[64:128, :])
                        nc.sync.dma_start(shuf[64:128, :], kb[0:64, :])
                        nc.vector.tensor_mul(ke, kb, csb)
                        nc.vector.tensor_mul(shuf, shuf, ssb)
                        nc.vector.tensor_add(ke, ke, shuf)

            # -------- P1b + P2: Q proj interleaved with attention ----------
            with tc.tile_pool(name="ap", bufs=1) as ap:
                a_sb = [ap.tile([128, NQ], MMDT, name=f"a{h}")
                        for h in range(NH)]
                with tc.tile_pool(name="p2", bufs=1) as p2, \
                     tc.tile_pool(name="p2s", bufs=2) as p2s, \
                     tc.tile_pool(name="p2w", bufs=10) as p2w, \
                     tc.tile_pool(name="qtp", bufs=2) as qtp, \
                     tc.tile_pool(name="att", bufs=5) as att, \
                     tc.tile_pool(name="atts", bufs=2) as atts:
                    cq = p2.tile([D, NQ], MMDT, name="cq")
                    sq = p2.tile([D, NQ], MMDT, name="sq")
                    nc.sync.dma_start(cq, cq_d)
                    nc.sync.dma_start(sq, sq_d)

                    for hg in range(4):
                        qT_sb = {}
                        with tc.tile_pool(name=f"qps{hg}", bufs=1,
                                          space="PSUM") as p2ps:
                            qps = [p2ps.tile([128, NQ], F32, name=f"qps{j}",
                                             bufs=1) for j in range(4)]
                            for c in range(HC):
                                wq = p2w.tile([128, NQ], MMDT, name="wq")
                                nc.sync.dma_start(
                                    wq, wqT[c * 128:(c + 1) * 128,
                                            hg * NQ:(hg + 1) * NQ])
                                for j in range(4):
                                    nc.tensor.matmul(
                                        qps[j], wq[:, j * 128:(j + 1) * 128],
                                        xq_sb[c], start=(c == 0),
                                        stop=(c == HC - 1))
                            for j in range(4):
                                h = hg * 4 + j
                                qT_sb[h] = qtp.tile([128, NQ], MMDT,
                                                    name=f"qT{j}")
                                qb = p2s.tile([128, NQ], MMDT, name="qb")
                                nc.scalar.activation(qb, qps[j], Ident,
                                                     bias=bqT[:, h:h + 1])
                                qe = qT_sb[h]
                                shufq = p2s.tile([128, NQ], MMDT,
                                                 name="shufq")
                                nc.sync.dma_start(shufq[0:64, :],
                                                  qb[64:128, :])
                                nc.sync.dma_start(shufq[64:128, :],
                                                  qb[0:64, :])
                                nc.vector.tensor_mul(qe, qb, cq)
                                nc.vector.tensor_mul(shufq, shufq, sq)
                                nc.vector.tensor_add(qe, qe, shufq)

                        if hg == 0:
                            for kt2 in range(KT2):
                                nc.sync.dma_start(
                                    mask_sb[kt2],
                                    maskT_d[256 * kt2:256 * (kt2 + 1),
                                            :].rearrange(
                                        "(a p) q -> p a q", a=2))
                        with tc.tile_pool(name=f"attps{hg}", bufs=1,
                                          space="PSUM") as attps:
                            for h in range(hg * 4, hg * 4 + 4):
                                g = h // (NH // NKV)
                                ops = attps.tile([128, NQ], F32, name="ops",
                                                 bufs=1)
                                stats = attps.tile([128, NQ], F32,
                                                   name="stats", bufs=1)
                                for kt2 in range(KT2):
                                    sps = attps.tile([128, 2, NQ], F32,
                                                     name="sps", bufs=3)
                                    ebuf = att.tile([128, 2, NQ], MMDT,
                                                    name="ebuf")
                                    for j2 in range(2):
                                        kt = 2 * kt2 + j2
                                        nc.tensor.matmul(
                                            sps[:, j2, :],
                                            kT_sb[g][:, kt * 128:
                                                     (kt + 1) * 128],
                                            qT_sb[h], start=True, stop=True)
                                    nc.scalar.activation(
                                        ebuf.rearrange("p a b -> p (a b)"),
                                        sps.rearrange("p a b -> p (a b)"),
                                        Exp, scale=float(SM_SCALE))
                                    nc.vector.tensor_mul(
                                        ebuf.rearrange("p a b -> p (a b)"),
                                        ebuf.rearrange("p a b -> p (a b)"),
                                        mask_sb[kt2].rearrange(
                                            "p a b -> p (a b)"))
                                    for j2 in range(2):
                                        kt = 2 * kt2 + j2
                                        nc.tensor.matmul(
                                            stats, ones_sq, ebuf[:, j2, :],
                                            start=(kt == 0),
                                            stop=(kt == KT - 1))
                                        nc.tensor.matmul(
                                            ops,
                                            v_sb[kt][:, g * D:(g + 1) * D],
                                            ebuf[:, j2, :],
                                            start=(kt == 0),
                                            stop=(kt == KT - 1))
                                recip = atts.tile([128, NQ], F32,
                                                  name="recip")
                                nc.vector.reciprocal_approx_fast(
                                    out=recip, in_=stats)
                                nc.vector.tensor_mul(a_sb[h], ops, recip)

                # ------------- P3: o_proj ------------------------------
                with tc.tile_pool(name="wop", bufs=1) as wop, \
                     tc.tile_pool(name="wos", bufs=3) as wos, \
                     tc.tile_pool(name="wops", bufs=1, space="PSUM") as wops:
                    for ec in range(4):
                        wo_t = [wop.tile([128, NQ], MMDT, name=f"wo{h}",
                                         bufs=2) for h in range(NH)]
                        for h in range(NH):
                            nc.sync.dma_start(
                                wo_t[h], woT[h * 128:(h + 1) * 128,
                                             ec * NQ:(ec + 1) * NQ])
                        for qs_ in range(QS):
                            opo = wops.tile([128, NQ], F32, name="opo",
                                            bufs=3)
                            for h in range(NH):
                                nc.tensor.matmul(
                                    opo,
                                    a_sb[h][:, qs_ * 128:(qs_ + 1) * 128],
                                    wo_t[h], start=(h == 0),
                                    stop=(h == NH - 1))
                            osb = wos.tile([128, NQ], F32, name="osb")
                            nc.vector.tensor_copy(osb, opo)
                            nc.sync.dma_start(
                                out_d[qs_ * 128:(qs_ + 1) * 128,
                                      ec * NQ:(ec + 1) * NQ], osb)
    return nc


def get_nc(key):
    if key not in _BUILD_CACHE:
        if key == "causal":
            nc = _build_causal()
        else:
            nc = _build_nc(key)
        nc.finalize()
        _BUILD_CACHE[key] = nc
    return _BUILD_CACHE[key]


def _host_prep(hidden_states, cos, sin, attention_mask, Wq, bq, Wk, bk, Wv,
               bv, Wo, mm="f32r"):
    f = np.float32
    if mm == "f32r":
        rnd = _round_fp32r
    else:
        def rnd(a):
            return np.ascontiguousarray(a, f)
    hs = np.asarray(hidden_states, f)
    cos = np.asarray(cos, f)
    sin = np.asarray(sin, f)
    mask = np.asarray(attention_mask, f)
    ar = np.arange(D)

    shared = {
        "wqT": rnd(np.asarray(Wq, f).T),
        "wkT": rnd(np.asarray(Wk, f).T),
        "wvT": rnd(np.asarray(Wv, f).T),
        "woT": rnd(np.asarray(Wo, f).T),
        "bqT": np.ascontiguousarray(np.asarray(bq, f).reshape(NH, D).T),
        "bkT": np.ascontiguousarray(np.asarray(bk, f).reshape(NKV, D).T),
        "bv": rnd(np.asarray(bv, f).reshape(1, NKV * D)),
    }

    per_batch = []
    for b in range(B):
        xT = rnd(hs[b].T)
        cosT = rnd(cos[_STREAM_IDX, b, :, ar])  # [128, S]
        sinT = rnd(sin[_STREAM_IDX, b, :, ar])
        sinT[0:64, :] *= -1.0   # rotate_half sign folded into sin
        maskT = rnd(np.exp(mask[b, 0].T.astype(np.float64)
                           ).astype(np.float32))
        per_batch.append((xT, cosT, sinT, maskT))

    in_maps = []
    for c in range(N_CORES):
        b, qc = divmod(c, N_CORES // B)
        xT, cosT, sinT, maskT = per_batch[b]
        qsl = slice(qc * NQ, (qc + 1) * NQ)
        order = [qc] + [o for o in range(N_CORES // B) if o != qc]
        tperm = np.concatenate([np.arange(o * NQ, (o + 1) * NQ)
                                for o in order])
        m = dict(shared)
        m["xT"] = np.ascontiguousarray(xT[:, tperm])
        m["cosT"] = np.ascontiguousarray(cosT[:, tperm])
        m["sinT"] = np.ascontiguousarray(sinT[:, tperm])
        m["maskT"] = np.ascontiguousarray(maskT[tperm][:, qsl])
        m["cosTq"] = np.ascontiguousarray(cosT[:, qsl])
        m["sinTq"] = np.ascontiguousarray(sinT[:, qsl])
        in_maps.append(m)
    return in_maps


def kernel(hidden_states, cos, sin, attention_mask, Wq, bq, Wk, bk, Wv, bv,
           Wo, _trace=False, _mm="causal"):
    from concourse.bass_utils import run_bass_kernel_spmd

    if _mm == "causal" and not _causal_mask_ok(attention_mask):
        _mm = "f32r"

    if _mm == "causal":
        in_maps = _host_prep_causal(hidden_states, cos, sin, Wq, bq, Wk, bk,
                                    Wv, bv, Wo)
        nc = get_nc("causal")
        res = run_bass_kernel_spmd(nc, in_maps, list(range(N_CORES)),
                                   trace=_trace)
        out = np.empty((B, S, HID), np.float32)
        for c in range(N_CORES):
            b, c4 = divmod(c, N_CORES // B)
            out[b, c4::4, :] = res.results[c]["out"]
        kernel._last_results = res
        return out

    in_maps = _host_prep(hidden_states, cos, sin, attention_mask, Wq, bq, Wk,
                         bk, Wv, bv, Wo, mm=_mm)
    nc = get_nc(_mm)
    res = run_bass_kernel_spmd(nc, in_maps, list(range(N_CORES)),
                               trace=_trace)
    out = np.empty((B, S, HID), np.float32)
    for c in range(N_CORES):
        b, qc = divmod(c, N_CORES // B)
        out[b, qc * NQ:(qc + 1) * NQ, :] = res.results[c]["out"]
    kernel._last_results = res
    return out
